# revision 1
# baseline (speedup 1.0000x reference)
"""Trainium2 Bass kernel for nn_MultiHeadAttention (B=2, S=2048, D=1024, H=16, causal).

Sharding across 8 NeuronCores (single SPMD program):
  - Core c owns batch b=c//4 and two 256-token query chunks {p, 7-p} (p=c%4)
    of that batch; the pairing balances causal attention work (every core
    covers 18 key-blocks of true work).
  - Phase 1: each core projects Q/K/V for its 512 tokens at full width.
    The 1/sqrt(64) score scale is folded into Wk/bk on the host.
  - Two AllGathers (replica groups [[0-3],[4-7]], i.e. per batch) publish
    K^T and V so that addresses are identical on every core.
  - Phase 2: streaming softmax in transposed layout scoresT[k, q] (no
    on-chip transposes); the softmax denominator falls out of an extra
    ones-column in the V operand of the attn@V matmul.  Causal masking and
    the per-core staircase use host-precomputed additive mask tiles, with
    uniform loop extents (8 blocks for the low chunk, 16 for the high one).
  - Phase 3: output projection for the core's own tokens only (row-parallel
    over tokens => no reduction); the host re-assembles the full output.
"""
import numpy as np

import concourse.bass as bass
import concourse.bacc as bacc
import concourse.mybir as mybir
import concourse.tile as tile
from concourse.bass_utils import run_bass_kernel_spmd
from concourse.tile_rust import add_dep_helper

B, S, D, H, HD = 2, 2048, 1024, 16, 64
NC = 8
P = 128
NEG = -1e10
F32 = mybir.dt.float32

# compute dtype for matmul-feeding tensors: float32r runs the PE at 4x the
# fp32 rate (1 cyc/row at N>=256); the BIR verifier requires the whole
# producer chain of a float32r matmul operand to be declared float32r.
CDT = mybir.dt.float32r

TRACE = False        # set True (e.g. from test.py) to capture an NTFF profile
LAST_RESULT = None   # BassKernelResults of the most recent kernel() call


def _mm(ap):
    return ap


def sel_tokens(p):
    return list(range(256 * p, 256 * p + 256)) + list(
        range(256 * (7 - p), 256 * (7 - p) + 256)
    )


def _kblk(j):
    """Original 128-token key block j -> (rank-in-group, column offset)."""
    q = j // 2
    rr = q if q <= 3 else 7 - q
    off = (0 if q <= 3 else 256) + 128 * (j % 2)
    return rr, off


def _emit(causal: bool, repeat: int = 1):
    nc = bacc.Bacc(trn_type="TRN2", num_devices=NC)
    ident = mybir.ActivationFunctionType.Identity
    fexp = mybir.ActivationFunctionType.Exp

    xT = nc.dram_tensor("xT", [D, 512], CDT, kind="ExternalInput")
    wqT = nc.dram_tensor("wqT", [D, D], CDT, kind="ExternalInput")
    wkT = nc.dram_tensor("wkT", [D, D], CDT, kind="ExternalInput")
    wvT = nc.dram_tensor("wvT", [D, D], CDT, kind="ExternalInput")
    woT = nc.dram_tensor("woT", [D, D], CDT, kind="ExternalInput")
    bq_d = nc.dram_tensor("bq", [P, 8], F32, kind="ExternalInput")
    bk_d = nc.dram_tensor("bk", [P, 8], F32, kind="ExternalInput")
    bv_d = nc.dram_tensor("bv", [1, D], CDT, kind="ExternalInput")
    bo_d = nc.dram_tensor("bo", [P, 8], F32, kind="ExternalInput")
    if causal:
        cmb_d = nc.dram_tensor("cmb", [P, 16, 256], F32, kind="ExternalInput")
    outT = nc.dram_tensor("outT", [D, 512], F32, kind="ExternalOutput")

    kt_loc = nc.dram_tensor("kt_loc", [D, 512], CDT)
    v_loc = nc.dram_tensor("v_loc", [512, D], CDT)
    kt_all = nc.dram_tensor("kt_all", [4 * D, 512], CDT)
    v_all = nc.dram_tensor("v_all", [4 * 512, D], CDT)

    with tile.TileContext(nc) as tc, \
         tc.tile_pool(name="const", bufs=1) as const, \
         tc.tile_pool(name="w", bufs=1) as wpool, \
         tc.tile_pool(name="big", bufs=1) as big, \
         tc.tile_pool(name="io", bufs=3) as io, \
         tc.tile_pool(name="kv", bufs=6) as kv, \
         tc.tile_pool(name="ex", bufs=4) as ex, \
         tc.tile_pool(name="sm", bufs=2) as sm, \
         tc.tile_pool(name="ps_big", bufs=2, space="PSUM") as ps_big, \
         tc.tile_pool(name="ps_sc", bufs=3, space="PSUM") as ps_sc, \
         tc.tile_pool(name="ps_ctx", bufs=2, space="PSUM") as ps_ctx, \
         tc.tile_pool(name="ps_rep", bufs=1, space="PSUM") as ps_rep:

        # ---------- constants ----------
        ones_f = const.tile([P, P], F32)
        nc.gpsimd.memset(ones_f[:], 1.0)
        ones = const.tile([P, P], CDT)
        nc.vector.tensor_copy(ones[:], ones_f[:])
        bq_sb = const.tile([P, 8], F32)
        nc.sync.dma_start(bq_sb[:], bq_d[:])
        bk_sb = const.tile([P, 8], F32)
        nc.sync.dma_start(bk_sb[:], bk_d[:])
        bv_sb = const.tile([1, D], CDT)
        nc.sync.dma_start(bv_sb[:], bv_d[:])
        bo_sb = const.tile([P, 8], F32)
        nc.sync.dma_start(bo_sb[:], bo_d[:])
        if causal:
            cmb_sb = big.tile([P, 16, 256], F32)
            nc.sync.dma_start(cmb_sb[:], cmb_d[:])

        for _rep in range(repeat):
            # ---------- phase 1: projections for this core's 512 tokens ----------
            xt_sb = big.tile([P, 8, 512], CDT)
            xr = xT.rearrange("(o p) t -> p o t", p=P)
            for _kt in range(8):
                nc.sync.dma_start(xt_sb[:, _kt, :], xr[:, _kt, :])
            qt_sb = big.tile([P, 8, 512], CDT)

            def proj_qk(w_dram, bias_sb, to_dram):
                w_sb = wpool.tile([P, 8, D], CDT, tag="w")
                nc.sync.dma_start(w_sb[:], w_dram.rearrange("(o p) t -> p o t", p=P))
                for dt in range(8):
                    pt = ps_big.tile([P, 512], F32)
                    for kt in range(8):
                        nc.tensor.matmul(
                            pt[:], _mm(w_sb[:, kt, 128 * dt:128 * dt + 128]),
                            _mm(xt_sb[:, kt, :]), start=(kt == 0), stop=(kt == 7))
                    if to_dram is None:
                        nc.scalar.activation(qt_sb[:, dt, :], pt[:], ident,
                                             bias=bias_sb[:, dt:dt + 1])
                    else:
                        t = io.tile([P, 512], CDT, tag="io")
                        nc.scalar.activation(t[:], pt[:], ident,
                                             bias=bias_sb[:, dt:dt + 1])
                        nc.sync.dma_start(
                            to_dram.rearrange("(o p) t -> p o t", p=P)[:, dt, :],
                            t[:])

            rg = [[0, 1, 2, 3], [4, 5, 6, 7]]

            proj_qk(wkT, bk_sb, kt_loc)
            # kick off the K AllGather while V/Q projections still run
            cc_k = nc.gpsimd.collective_compute(
                "AllGather", mybir.AluOpType.bypass, replica_groups=rg,
                ins=[kt_loc[:]], outs=[kt_all[:]])

            wv_sb = wpool.tile([P, 8, D], CDT, tag="w")
            wvr = wvT.rearrange("(o p) t -> p o t", p=P)
            for _kt in range(8):
                nc.sync.dma_start(wv_sb[:, _kt, :], wvr[:, _kt, :])
            for st in range(4):
                for hf in range(2):
                    pt = ps_big.tile([P, 512], F32)
                    for kt in range(8):
                        nc.tensor.matmul(
                            pt[:], _mm(xt_sb[:, kt, 128 * st:128 * st + 128]),
                            _mm(wv_sb[:, kt, 512 * hf:512 * hf + 512]),
                            start=(kt == 0), stop=False)
                    nc.tensor.matmul(
                        pt[:], _mm(ones[0:1, 0:P]),
                        _mm(bv_sb[0:1, 512 * hf:512 * hf + 512]),
                        start=False, stop=True)
                    t = io.tile([P, 512], CDT, tag="io")
                    nc.scalar.copy(t[:], pt[:])
                    nc.sync.dma_start(
                        v_loc[128 * st:128 * st + 128, 512 * hf:512 * hf + 512],
                        t[:])
            cc_v = nc.gpsimd.collective_compute(
                "AllGather", mybir.AluOpType.bypass, replica_groups=rg,
                ins=[v_loc[:]], outs=[v_all[:]])

            proj_qk(wqT, bq_sb, None)
            wo_sb = wpool.tile([P, 8, D], CDT, tag="w")
            nc.sync.dma_start(wo_sb[:],
                              woT.rearrange("(o p) t -> p o t", p=P))

            # ---------- phase 2: attention ----------
            # Single merged pass: key-blocks 0..7 are valid for BOTH q-chunks
            # (role 1's true extent is always >= 10), so supers 0..1 process
            # them once at N=512 across both chunks; supers 2..3 (blocks
            # 8..15) touch only the high chunk (cols 256:512) when causal.
            # kv blocks are loaded once per super and shared by all heads;
            # per-head ctx accumulates in PSUM within a super and in SBUF
            # (acc, row 64 = softmax denominator) across supers.
            ctx_sb = big.tile([P, 8, 512], CDT)
            acc = big.tile([P, H, 512], F32)
            for sj in range(4):
                wid = 512 if (not causal or sj < 2) else 256
                qoff = 0 if (not causal or sj < 2) else 256
                kts, vas = [], []
                for jj in range(4):
                    j = 4 * sj + jj
                    rr, off = _kblk(j)
                    kt_t = kv.tile([P, 8, 128], CDT, tag="kt")
                    d1 = nc.sync.dma_start(
                        kt_t[:],
                        kt_all.rearrange("(r o p) t -> p r o t", p=P, o=8)
                        [:, rr, :, off:off + 128])
                    add_dep_helper(d1.ins, cc_k.ins, reason="read after AG-K")
                    va = kv.tile([P, H, 66], CDT, tag="va")
                    d2 = nc.sync.dma_start(
                        va[:, :, 1:65],
                        v_all[512 * rr + off:512 * rr + off + 128, :]
                        .rearrange("p (h d) -> p h d", h=H))
                    add_dep_helper(d2.ins, cc_v.ins, reason="read after AG-V")
                    nc.vector.tensor_copy(va[:, :, 65:66],
                                          ones[:, 0:H, None])
                    kts.append(kt_t)
                    vas.append(va)
                for h in range(H):
                    hb = 64 * (h % 2)
                    ctx_ps = ps_ctx.tile([P, 512], F32)
                    for jj in range(4):
                        j = 4 * sj + jj
                        sc = ps_sc.tile([P, 512], F32)
                        nc.tensor.matmul(
                            sc[:, 0:wid],
                            _mm(kts[jj][hb:hb + 64, h // 2, :]),
                            _mm(qt_sb[hb:hb + 64, h // 2,
                                      qoff:qoff + wid]),
                            start=True, stop=True)
                        if causal:
                            # mask the low (sj<2: role-0) / high (sj>=2:
                            # role-1) chunk's 256 columns of this block
                            nc.vector.tensor_tensor(
                                sc[:, 0:256], sc[:, 0:256], cmb_sb[:, j, :],
                                mybir.AluOpType.add)
                        et = ex.tile([P, 512], CDT, tag="exp")
                        nc.scalar.activation(et[:, 0:wid], sc[:, 0:wid], fexp)
                        # ctx rows 0:64, softmax-denominator row at 64
                        nc.tensor.matmul(ctx_ps[0:65, 0:wid],
                                         _mm(vas[jj][:, h, 1:66]),
                                         _mm(et[:, 0:wid]), start=(jj == 0),
                                         stop=(jj == 3))
                    if sj == 0:
                        nc.vector.tensor_copy(acc[0:65, h, :],
                                              ctx_ps[0:65, :])
                    else:
                        nc.vector.tensor_tensor(
                            acc[0:65, h, qoff:qoff + wid],
                            ctx_ps[0:65, 0:wid],
                            acc[0:65, h, qoff:qoff + wid],
                            mybir.AluOpType.add)
            # normalize + output-project one q-half (cols off:off+w).
            # When causal, the low half is final after supers 0..1, so its
            # tail + projection overlap supers 2..3.
            def norm_and_proj(off, w, wo_sb):
                for h in range(H):
                    recip_sb = sm.tile([P, 512], CDT, tag="recip")
                    with nc.allow_low_precision(
                            reason="softmax denom in f32r"):
                        nc.vector.reciprocal(recip_sb[64:65, off:off + w],
                                             acc[64:65, h, off:off + w])
                    rep_ps = ps_rep.tile([P, 512], F32)
                    nc.tensor.matmul(rep_ps[0:64, 0:w],
                                     _mm(ones[64:65, 0:64]),
                                     _mm(recip_sb[64:65, off:off + w]),
                                     start=True, stop=True)
                    rep_sb = sm.tile([P, 512], F32, tag="rep")
                    nc.scalar.copy(rep_sb[0:64, 0:w], rep_ps[0:64, 0:w])
                    if h % 2 == 0:
                        nc.vector.tensor_tensor(
                            ctx_sb[0:64, h // 2, off:off + w],
                            acc[0:64, h, off:off + w],
                            rep_sb[0:64, 0:w], mybir.AluOpType.mult)
                    else:
                        tmp = sm.tile([P, 512], CDT, tag="ctxtmp")
                        nc.vector.tensor_tensor(
                            tmp[0:64, 0:w], acc[0:64, h, off:off + w],
                            rep_sb[0:64, 0:w], mybir.AluOpType.mult)
                        nc.sync.dma_start(
                            ctx_sb[64:128, h // 2, off:off + w],
                            tmp[0:64, 0:w])
                for m in range(8):
                    pt = ps_big.tile([P, 512], F32)
                    for kt in range(8):
                        nc.tensor.matmul(
                            pt[:, 0:w],
                            _mm(wo_sb[:, kt, 128 * m:128 * m + 128]),
                            _mm(ctx_sb[:, kt, off:off + w]),
                            start=(kt == 0), stop=(kt == 7))
                    t = io.tile([P, 512], F32, tag="io")
                    nc.scalar.activation(t[:, 0:w], pt[:, 0:w], ident,
                                         bias=bo_sb[:, m:m + 1])
                    nc.sync.dma_start(
                        outT.rearrange("(o p) t -> p o t", p=P)
                        [:, m, off:off + w], t[:, 0:w])

            if causal:
                norm_and_proj(0, 256, wo_sb)
                norm_and_proj(256, 256, wo_sb)
            else:
                norm_and_proj(0, 512, wo_sb)

    nc.compile()
    return nc


_CACHE = {}


def _get_nc(causal: bool, repeat: int = 1):
    key = (causal, repeat)
    if key not in _CACHE:
        _CACHE[key] = _emit(causal, repeat)
    return _CACHE[key]


def _mask_tiles(p):
    """Per-core additive mask [128, 16, 256] for causal staircase."""
    k = np.arange(128)[:, None]
    c = np.arange(256)[None, :]
    m1 = np.where(c - k >= 0, 0.0, NEG).astype(np.float32)
    m2 = np.where(c - 128 - k >= 0, 0.0, NEG).astype(np.float32)
    cmb = np.zeros((128, 16, 256), dtype=np.float32)
    # slots 0..7: role 0 (chunk p), true extent 2p+2
    for j in range(8):
        if j == 2 * p:
            cmb[:, j, :] = m1
        elif j == 2 * p + 1:
            cmb[:, j, :] = m2
        elif j > 2 * p + 1:
            cmb[:, j, :] = NEG
    # slots 8..15: role 1 (chunk 7-p), true extent 16-2p; blocks 0..7 unmasked
    for j in range(8, 16):
        if j == 14 - 2 * p:
            cmb[:, j, :] = m1
        elif j == 15 - 2 * p:
            cmb[:, j, :] = m2
        elif j > 15 - 2 * p:
            cmb[:, j, :] = NEG
    return cmb


def kernel(**inputs):
    x = np.ascontiguousarray(np.asarray(inputs["x"], dtype=np.float32))
    Wq = np.asarray(inputs["Wq"], dtype=np.float32)
    bq = np.asarray(inputs["bq"], dtype=np.float32)
    Wk = np.asarray(inputs["Wk"], dtype=np.float32)
    bk = np.asarray(inputs["bk"], dtype=np.float32)
    Wv = np.asarray(inputs["Wv"], dtype=np.float32)
    bv = np.asarray(inputs["bv"], dtype=np.float32)
    Wo = np.asarray(inputs["Wo"], dtype=np.float32)
    bo = np.asarray(inputs["bo"], dtype=np.float32)
    causal = bool(int(np.asarray(inputs["enable_causal"])))

    scale = np.float32(1.0 / np.sqrt(HD))
    wqT = np.ascontiguousarray(Wq.T)
    wkT = np.ascontiguousarray((Wk * scale).T)
    wvT = np.ascontiguousarray(Wv.T)
    woT = np.ascontiguousarray(Wo.T)
    bqt = np.ascontiguousarray(bq.reshape(8, P).T)
    bkt = np.ascontiguousarray((bk * scale).reshape(8, P).T)
    bvr = np.ascontiguousarray(bv.reshape(1, D))
    bot = np.ascontiguousarray(bo.reshape(8, P).T)

    nc = _get_nc(causal)
    in_maps = []
    for c in range(NC):
        b, p = divmod(c, 4)
        sel = sel_tokens(p)
        xTc = np.ascontiguousarray(x[b][sel, :].T)
        m = {"xT": xTc, "wqT": wqT, "wkT": wkT, "wvT": wvT, "woT": woT,
             "bq": bqt, "bk": bkt, "bv": bvr, "bo": bot}
        if causal:
            m["cmb"] = _mask_tiles(p)
        in_maps.append(m)

    global LAST_RESULT
    res = run_bass_kernel_spmd(nc, in_maps, list(range(NC)), trace=TRACE)
    LAST_RESULT = res
    out = np.empty((B, S, D), dtype=np.float32)
    for c in range(NC):
        b, p = divmod(c, 4)
        sel = sel_tokens(p)
        out[b, sel, :] = res.results[c]["outT"].T
    return out



# revision 12
# speedup vs baseline: 3.6762x; 3.6762x over previous
"""Trainium2 Bass kernel for nn_MultiHeadAttention (B=2, S=2048, D=1024, H=16, causal).

Sharding across 8 NeuronCores (single SPMD program, head-parallel TP):
  - Core c owns batch b=c//4 and head group g=c%4 (4 heads = 256 of the 1024
    projection columns).  W_q/W_k/W_v are column-sharded, W_o row-sharded.
  - Each core projects Q/K/V for ALL 2048 tokens of its batch but only its 4
    heads, runs full causal attention for those heads entirely in SBUF (no
    K/V exchange => ZERO collectives), then computes its partial output
    projection out_partial = ctx_heads @ Wo_rows.  The host unshard step sums
    the 4 partial outputs per batch and adds bo (the row-parallel reduction
    of tensor-parallel attention, folded into the host-side gather that the
    full-IO contract already requires).
  - bf16 operands everywhere on the PE (1 cycle/row at any N); f32 PSUM
    accumulation; 1/sqrt(64) folded into Wk/bk on the host.
  - Attention is computed in transposed score layout scoresT[k, q] per
    128-key block x 256-query chunk, all 4 heads batched into one
    [128, 1024] PSUM tile so a single Act exp covers them.  The softmax
    denominator falls out of a ones-column appended to V (65-row ctx
    matmuls); causal masking is multiplicative on the exp'd tile (bf16,
    diagonal blocks only).  Loop extents are causally tight and identical
    on every core (each core sees the same (qc, kb) staircase).
  - Projections and attention are software-pipelined: K/V/Q are projected a
    512-token chunk at a time, with the attention q-chunks interleaved so
    the Act-engine exp work spreads across the whole timeline.
"""
import numpy as np
import ml_dtypes

import concourse.bass as bass
import concourse.bacc as bacc
import concourse.mybir as mybir
import concourse.tile as tile
from concourse.bass_utils import run_bass_kernel_spmd

B, S, D, H, HD = 2, 2048, 1024, 16, 64
NC = 8
P = 128
F32 = mybir.dt.float32
BF16 = mybir.dt.bfloat16
NPBF = ml_dtypes.bfloat16

TRACE = False        # set True (e.g. from test.py) to capture an NTFF profile
LAST_RESULT = None   # BassKernelResults of the most recent kernel() call


def _emit(causal: bool):
    nc = bacc.Bacc(trn_type="TRN2", num_devices=NC)
    fexp = mybir.ActivationFunctionType.Exp
    mult = mybir.AluOpType.mult

    # ---- per-core DRAM inputs (host pre-sharded / pre-transposed) ----
    xT4 = nc.dram_tensor("xT4", [4, P, 8, 512], BF16, kind="ExternalInput")
    wq_d = nc.dram_tensor("wq", [P, 8, 256], BF16, kind="ExternalInput")
    wk_d = nc.dram_tensor("wk", [P, 8, 256], BF16, kind="ExternalInput")
    wv_d = nc.dram_tensor("wv", [P, 8, 256], BF16, kind="ExternalInput")
    wo_d = nc.dram_tensor("wo", [P, 2, 1024], BF16, kind="ExternalInput")
    bq_d = nc.dram_tensor("bq", [P, 2], F32, kind="ExternalInput")
    bk_d = nc.dram_tensor("bk", [P, 2], F32, kind="ExternalInput")
    bv_d = nc.dram_tensor("bv", [1, 256], BF16, kind="ExternalInput")
    id_d = nc.dram_tensor("ident", [64, 64], BF16, kind="ExternalInput")
    if causal:
        m0_d = nc.dram_tensor("m0", [P, 1024], BF16, kind="ExternalInput")
        m1_d = nc.dram_tensor("m1", [P, 1024], BF16, kind="ExternalInput")
    outT = nc.dram_tensor("outT", [D, S], F32, kind="ExternalOutput")

    with tile.TileContext(nc) as tc, \
         tc.tile_pool(name="const", bufs=1) as const, \
         tc.tile_pool(name="w", bufs=1) as wpool, \
         tc.tile_pool(name="big", bufs=1) as big, \
         tc.tile_pool(name="et", bufs=3) as etp, \
         tc.tile_pool(name="sm", bufs=2) as smp, \
         tc.tile_pool(name="ob", bufs=2) as obp, \
         tc.tile_pool(name="ps_sc", bufs=2, space="PSUM") as ps_sc, \
         tc.tile_pool(name="ps_ctx", bufs=2, space="PSUM") as ps_ctx, \
         tc.tile_pool(name="ps_aux", bufs=2, space="PSUM") as ps_aux:

        # ---------- constants / weights ----------
        ones_sb = const.tile([P, P], BF16)
        nc.gpsimd.memset(ones_sb[:], 1.0)
        id_sb = const.tile([64, 64], BF16)
        nc.sync.dma_start(id_sb[:], id_d[:])
        bq_sb = const.tile([P, 2], F32)
        nc.sync.dma_start(bq_sb[:], bq_d[:])
        bk_sb = const.tile([P, 2], F32)
        nc.sync.dma_start(bk_sb[:], bk_d[:])
        bv_sb = const.tile([1, 256], BF16)
        nc.sync.dma_start(bv_sb[:], bv_d[:])
        if causal:
            m0_sb = const.tile([P, 1024], BF16)
            nc.sync.dma_start(m0_sb[:], m0_d[:])
            m1_sb = const.tile([P, 1024], BF16)
            nc.sync.dma_start(m1_sb[:], m1_d[:])

        wk_sb = wpool.tile([P, 8, 256], BF16)
        nc.sync.dma_start(wk_sb[:], wk_d[:])
        wv_sb = wpool.tile([P, 8, 256], BF16)
        nc.sync.dma_start(wv_sb[:], wv_d[:])
        wq_sb = wpool.tile([P, 8, 256], BF16)
        nc.sync.dma_start(wq_sb[:], wq_d[:])
        wo_sb = wpool.tile([P, 2, 1024], BF16)
        nc.sync.dma_start(wo_sb[:], wo_d[:])

        xt_sb = big.tile([P, 8, 2048], BF16)
        for nch in range(4):
            nc.sync.dma_start(xt_sb[:, :, 512 * nch:512 * nch + 512],
                              xT4[nch, :, :, :])

        qt_sb = big.tile([P, 2, 2048], BF16)    # [hd-of-pair, hp, tokens]
        kt_sb = big.tile([P, 2, 2048], BF16)
        va_sb = big.tile([P, 16, 260], BF16)    # [key, kb, 4x(64 v + 1 one)]
        ctxT_sb = big.tile([P, 2, 2048], BF16)  # [hd-of-pair, hp, tokens]
        nc.gpsimd.memset(
            va_sb.rearrange("p k (s c) -> p k s c", c=65)[:, :, :, 64:65], 1.0)

        # ---------- phase helpers ----------
        def proj_qk(w_sb, b_sb, dst_sb, nch):
            """Project one 512-token chunk of Q or K into [hd, tok] layout."""
            for hp in range(2):
                pt = ps_sc.tile([P, 512], F32, tag="sc")
                for kt in range(8):
                    nc.tensor.matmul(
                        pt[:], w_sb[:, kt, 128 * hp:128 * hp + 128],
                        xt_sb[:, kt, 512 * nch:512 * nch + 512],
                        start=(kt == 0), stop=(kt == 7))
                nc.vector.tensor_scalar_add(
                    dst_sb[:, hp, 512 * nch:512 * nch + 512], pt[:],
                    b_sb[:, hp:hp + 1])

        def proj_v(tt):
            """Project one 128-token tile of V into va layout [key, hd]."""
            pt = ps_sc.tile([P, 256], F32, tag="sc")
            for kt in range(8):
                nc.tensor.matmul(
                    pt[:], xt_sb[:, kt, 128 * tt:128 * tt + 128],
                    wv_sb[:, kt, :], start=(kt == 0), stop=False)
            nc.tensor.matmul(pt[:], ones_sb[0:1, 0:P], bv_sb[:],
                             start=False, stop=True)
            nc.vector.tensor_copy(
                va_sb.rearrange("p k (s c) -> p k s c", c=65)
                [:, tt, :, 0:64],
                pt.rearrange("p (s c) -> p s c", c=64))

        def attn(qc, norm=True):
            """One 256-query chunk: scores+softmax+ctx for all 4 heads."""
            import os as _os
            _kmask = int(_os.environ.get("KMASK", "1"))
            _kctx = int(_os.environ.get("KCTX", "1"))
            nkb = 2 * qc + 2 if causal else 16
            ctx = [ps_ctx.tile([P, 512], F32, tag="ctx", name=f"ctx{qc}_{hp}")
                   for hp in range(2)]
            # column slot per head: parity-grouped so each PSUM bank only
            # ever sees one PE tile row position (HW constraint: matmuls
            # into one bank must share the same partition base)
            SCOL = {0: 0, 2: 256, 1: 512, 3: 768}
            for kb in range(nkb):
                sc = ps_sc.tile([P, 1024], F32, tag="sc", name=f"sc{qc}_{kb}")
                for s in range(4):
                    hb, hp = 64 * (s % 2), s // 2
                    nc.tensor.matmul(
                        sc[:, SCOL[s]:SCOL[s] + 256],
                        kt_sb[hb:hb + 64, hp, 128 * kb:128 * kb + 128],
                        qt_sb[hb:hb + 64, hp, 256 * qc:256 * qc + 256],
                        start=True, stop=True)
                et = etp.tile([P, 1024], BF16, tag="et", name=f"et{qc}_{kb}")
                nc.scalar.activation(et[:], sc[:], fexp)
                if causal and kb >= 2 * qc and _kmask:
                    msk = m0_sb if kb == 2 * qc else m1_sb
                    nc.vector.tensor_tensor(et[:], et[:], msk[:], mult)
                if not _kctx:
                    continue
                for s in range(4):
                    # one accumulation group per PSUM bank: start only on the
                    # very first matmul into the bank, stop on the very last
                    hp = s // 2
                    nc.tensor.matmul(
                        ctx[hp][0:65, 256 * (s % 2):256 * (s % 2) + 256],
                        va_sb[:, kb, 65 * s:65 * s + 65],
                        et[:, SCOL[s]:SCOL[s] + 256],
                        start=(kb == 0 and s % 2 == 0),
                        stop=(kb == nkb - 1 and s % 2 == 1))
            # normalize + pack into ctxT (even head -> partitions 0:64,
            # odd head -> 64:128 via identity-matmul partition shift)
            if not norm:
                if _kctx:
                    for hp in range(2):
                        junk = smp.tile([P, 512], F32, tag="junk")
                        nc.vector.tensor_copy(junk[0:65, :], ctx[hp][0:65, :])
                return
            for hp in range(2):
                recip = smp.tile([P, 512], BF16, tag="recip")
                with nc.allow_low_precision(reason="softmax denom in bf16"):
                    nc.vector.reciprocal(recip[64:65, :], ctx[hp][64:65, :])
                rep_ps = ps_aux.tile([64, 512], F32, tag="aux",
                                     name=f"rep{qc}_{hp}")
                nc.tensor.matmul(rep_ps[0:64, :], ones_sb[64:65, 0:64],
                                 recip[64:65, :], start=True, stop=True)
                rep_sb = smp.tile([64, 512], BF16, tag="rep")
                nc.vector.tensor_copy(rep_sb[:], rep_ps[:])
                nc.vector.tensor_tensor(
                    ctxT_sb[0:64, hp, 256 * qc:256 * qc + 256],
                    ctx[hp][0:64, 0:256], rep_sb[0:64, 0:256], mult)
                tmp = smp.tile([64, 256], BF16, tag="tmp")
                nc.vector.tensor_tensor(
                    tmp[:], ctx[hp][0:64, 256:512], rep_sb[0:64, 256:512],
                    mult)
                sh = ps_aux.tile([P, 256], F32, tag="aux",
                                 name=f"sh{qc}_{hp}")
                nc.tensor.matmul(sh[64:128, :], id_sb[:], tmp[:],
                                 start=True, stop=True)
                nc.scalar.copy(ctxT_sb[64:128, hp, 256 * qc:256 * qc + 256],
                               sh[64:128, :])

        def oproj(qc):
            """Partial output projection for one 256-token chunk."""
            ob = obp.tile([P, 8, 256], F32, tag="ob")
            for m in range(8):
                po = ps_aux.tile([P, 256], F32, tag="aux",
                                 name=f"po{qc}_{m}")
                for kt in range(2):
                    nc.tensor.matmul(
                        po[:], wo_sb[:, kt, 128 * m:128 * m + 128],
                        ctxT_sb[:, kt, 256 * qc:256 * qc + 256],
                        start=(kt == 0), stop=(kt == 1))
                if m % 2 == 0:
                    nc.scalar.copy(ob[:, m, :], po[:])
                else:
                    nc.vector.tensor_copy(ob[:, m, :], po[:])
            nc.sync.dma_start(
                outT.rearrange("(m p) t -> p m t", p=P)
                [:, :, 256 * qc:256 * qc + 256], ob[:])

        # ---------- schedule ----------
        import os as _os
        _kn = int(_os.environ.get("KN", "4"))        # nch groups to emit
        _kattn = int(_os.environ.get("KATTN", "1"))  # emit attention?
        _knorm = int(_os.environ.get("KNORM", "1"))  # emit normalize?
        _koproj = int(_os.environ.get("KOPROJ", "1"))
        if _kn != 4 or not (_kattn and _knorm and _koproj):
            for nch in range(_kn):
                proj_qk(wk_sb, bk_sb, kt_sb, nch)
                for tt in range(4 * nch, 4 * nch + 4):
                    proj_v(tt)
                proj_qk(wq_sb, bq_sb, qt_sb, nch)
                if _kattn:
                    for qc in (2 * nch, 2 * nch + 1):
                        attn(qc, norm=_knorm)
                        if _koproj:
                            oproj(qc)
            # make sure outT is written so outputs bind
            ob = obp.tile([P, 8, 256], F32, tag="ob")
            nc.gpsimd.memset(ob[:], 0.0)
            nc.sync.dma_start(
                outT.rearrange("(m p) t -> p m t", p=P)[:, :, 1792:2048], ob[:])
        elif causal:
            # interleave projection chunks with the attention q-chunks they
            # unblock, so exp (Act) work spreads across the whole timeline
            for nch in range(4):
                proj_qk(wk_sb, bk_sb, kt_sb, nch)
                for tt in range(4 * nch, 4 * nch + 4):
                    proj_v(tt)
                proj_qk(wq_sb, bq_sb, qt_sb, nch)
                for qc in (2 * nch, 2 * nch + 1):
                    attn(qc)
                    oproj(qc)
        else:
            for nch in range(4):
                proj_qk(wk_sb, bk_sb, kt_sb, nch)
                for tt in range(4 * nch, 4 * nch + 4):
                    proj_v(tt)
                proj_qk(wq_sb, bq_sb, qt_sb, nch)
            for qc in range(8):
                attn(qc)
                oproj(qc)

    nc.compile()
    return nc


_CACHE = {}


def _get_nc(causal: bool):
    key = bool(causal)
    if key not in _CACHE:
        _CACHE[key] = _emit(key)
    return _CACHE[key]


def _masks():
    """Multiplicative causal masks for the two diagonal 128-key blocks of a
    256-query chunk, replicated across the 4 head slots."""
    i = np.arange(128)[:, None]
    j = np.arange(256)[None, :]
    m0 = (j >= i).astype(np.float32)          # keys [0:128) of the chunk
    m1 = (j - 128 >= i).astype(np.float32)    # keys [128:256)
    return (np.tile(m0, (1, 4)).astype(NPBF),
            np.tile(m1, (1, 4)).astype(NPBF))


def kernel(**inputs):
    x = np.asarray(inputs["x"], dtype=np.float32)
    Wq = np.asarray(inputs["Wq"], dtype=np.float32)
    bq = np.asarray(inputs["bq"], dtype=np.float32)
    Wk = np.asarray(inputs["Wk"], dtype=np.float32)
    bk = np.asarray(inputs["bk"], dtype=np.float32)
    Wv = np.asarray(inputs["Wv"], dtype=np.float32)
    bv = np.asarray(inputs["bv"], dtype=np.float32)
    Wo = np.asarray(inputs["Wo"], dtype=np.float32)
    bo = np.asarray(inputs["bo"], dtype=np.float32)
    causal = bool(int(np.asarray(inputs["enable_causal"])))

    scale = np.float32(1.0 / np.sqrt(HD))
    wqT = Wq.T                    # [in, out]
    wkT = (Wk * scale).T
    wvT = Wv.T
    woT = Wo.T                    # [ctx-dim, out]
    bks = bk * scale

    # per-batch x, transposed and chunked: [nch, p, o, t]
    xs = []
    for b in range(B):
        xt = x[b].T.reshape(8, 128, 4, 512).transpose(2, 1, 0, 3)
        xs.append(np.ascontiguousarray(xt.astype(NPBF)))

    ident = np.eye(64, dtype=NPBF)
    if causal:
        m0, m1 = _masks()

    nc = _get_nc(causal)
    in_maps = []
    for c in range(NC):
        b, g = divmod(c, 4)
        cols = slice(256 * g, 256 * g + 256)
        wq_r = np.ascontiguousarray(
            wqT[:, cols].reshape(8, 128, 256).transpose(1, 0, 2)).astype(NPBF)
        wk_r = np.ascontiguousarray(
            wkT[:, cols].reshape(8, 128, 256).transpose(1, 0, 2)).astype(NPBF)
        wv_r = np.ascontiguousarray(
            wvT[:, cols].reshape(8, 128, 256).transpose(1, 0, 2)).astype(NPBF)
        wo_r = np.ascontiguousarray(
            woT[cols, :].reshape(2, 128, 1024).transpose(1, 0, 2)).astype(NPBF)
        m = {"xT4": xs[b],
             "wq": wq_r, "wk": wk_r, "wv": wv_r, "wo": wo_r,
             "bq": np.ascontiguousarray(bq[cols].reshape(2, 128).T),
             "bk": np.ascontiguousarray(bks[cols].reshape(2, 128).T),
             "bv": np.ascontiguousarray(bv[cols].reshape(1, 256)).astype(NPBF),
             "ident": ident}
        if causal:
            m["m0"] = m0
            m["m1"] = m1
        in_maps.append(m)

    global LAST_RESULT
    res = run_bass_kernel_spmd(nc, in_maps, list(range(NC)), trace=TRACE)
    LAST_RESULT = res

    # unshard: sum the 4 head-group partials per batch (row-parallel Wo), +bo
    out = np.empty((B, S, D), dtype=np.float32)
    for b in range(B):
        acc = res.results[4 * b]["outT"].astype(np.float32)
        for g in range(1, 4):
            acc = acc + res.results[4 * b + g]["outT"]
        out[b] = acc.T + bo[None, :]
    return out


# revision 33
# speedup vs baseline: 5.5926x; 1.5213x over previous
"""Trainium2 Bass kernel for nn_MultiHeadAttention (B=2, S=2048, D=1024, H=16, causal).

Sharding across 8 NeuronCores (single SPMD program, head-parallel TP):
  - Core c owns batch b=c//4 and head group g=c%4 (4 heads = 256 of the 1024
    projection columns).  W_q/W_k/W_v are column-sharded, W_o row-sharded.
  - Each core projects Q/K/V for ALL 2048 tokens of its batch but only its 4
    heads, runs full causal attention for those heads entirely in SBUF (no
    K/V exchange => ZERO collectives), then computes its partial output
    projection out_partial = ctx_heads @ Wo_rows.  The host unshard step sums
    the 4 partial outputs per batch and adds bo (the row-parallel reduction
    of tensor-parallel attention, folded into the host-side gather that the
    full-IO contract already requires).
  - bf16 operands everywhere on the PE (1 cycle/row at any N); f32 PSUM
    accumulation; 1/sqrt(64) folded into Wk/bk on the host.
  - Attention runs as a flat software-pipelined stream of (qb, kb-pair)
    steps at 128-query granularity: transposed scores scoresT[k, q] for all
    4 heads of two key-blocks land in one [128,1024] PSUM tile (parity-
    grouped so each PSUM bank only sees one PE tile row position - HW
    constraint), a single Act exp covers the pair, causal masking is
    multiplicative on the exp'd tile (diagonal blocks only), and the
    context matmuls are q-major (stationary = exp tile, moving = V plus a
    ones-column that yields the softmax denominator as column 64).  The
    denominator is then a per-partition scalar, so normalization is plain
    tensor_scalar multiplies; a PE transpose packs the normalized context
    back to hd-major for the output projection.
  - ctx matmuls lag the score stream (software pipelining) and the
    normalize/transpose/output-projection work is spread as filler between
    later steps, so PE, Act and DVE stay concurrently busy.
"""
import numpy as np
import ml_dtypes

import concourse.bass as bass
import concourse.bacc as bacc
import concourse.mybir as mybir
import concourse.tile as tile
from concourse.bass_utils import run_bass_kernel_spmd

B, S, D, H, HD = 2, 2048, 1024, 16, 64
NC = 8
P = 128
F32 = mybir.dt.float32
BF16 = mybir.dt.bfloat16
NPBF = ml_dtypes.bfloat16

TRACE = False        # set True (e.g. from test.py) to capture an NTFF profile
LAST_RESULT = None   # BassKernelResults of the most recent kernel() call

LAG = 3              # ctx stream lags the score stream by this many pair-steps
K_FILL = 3           # filler items drained per pair-step


def _col(u, s):
    """Column of head-slot s, pair-position u in the [128,1024] score tile.

    Parity-grouped: bank 0 (cols 0:512) holds even heads (PE row base 0),
    bank 1 (cols 512:1024) odd heads (row base 64) - matmuls into one PSUM
    bank must share a single PE tile row position.
    """
    return (0 if s % 2 == 0 else 512) + 256 * u + (128 if s >= 2 else 0)


def _emit(causal: bool):
    nc = bacc.Bacc(trn_type="TRN2", num_devices=NC)
    fexp = mybir.ActivationFunctionType.Exp
    mult = mybir.AluOpType.mult

    # ---- per-core DRAM inputs (host pre-sharded / pre-transposed) ----
    xT4 = nc.dram_tensor("xT4", [4, P, 8, 512], BF16, kind="ExternalInput")
    wq_d = nc.dram_tensor("wq", [P, 8, 256], BF16, kind="ExternalInput")
    wk_d = nc.dram_tensor("wk", [P, 8, 256], BF16, kind="ExternalInput")
    wv_d = nc.dram_tensor("wv", [P, 8, 256], BF16, kind="ExternalInput")
    wo_d = nc.dram_tensor("wo", [P, 2, 1024], BF16, kind="ExternalInput")
    bq_d = nc.dram_tensor("bq", [P, 2], F32, kind="ExternalInput")
    bk_d = nc.dram_tensor("bk", [P, 2], F32, kind="ExternalInput")
    bv_d = nc.dram_tensor("bv", [1, 256], BF16, kind="ExternalInput")
    id_d = nc.dram_tensor("ident", [P, P], BF16, kind="ExternalInput")
    if causal:
        md_d = nc.dram_tensor("md", [P, 256], BF16, kind="ExternalInput")
    outT = nc.dram_tensor("outT", [D, S], F32, kind="ExternalOutput")

    with tile.TileContext(nc) as tc, \
         tc.tile_pool(name="const", bufs=1) as const, \
         tc.tile_pool(name="w", bufs=1) as wpool, \
         tc.tile_pool(name="big", bufs=1) as big, \
         tc.tile_pool(name="et", bufs=5) as etp, \
         tc.tile_pool(name="cq", bufs=2) as cqp, \
         tc.tile_pool(name="ob", bufs=2) as obp, \
         tc.tile_pool(name="ps_sc", bufs=2, space="PSUM") as ps_sc, \
         tc.tile_pool(name="ps_ctx", bufs=2, space="PSUM") as ps_ctx, \
         tc.tile_pool(name="ps_aux", bufs=2, space="PSUM") as ps_aux:

        # ---------- constants / weights (critical-path DMA order) ----------
        wk_sb = wpool.tile([P, 8, 256], BF16)
        nc.sync.dma_start(wk_sb[:], wk_d[:])
        bk_sb = const.tile([P, 2], F32)
        nc.sync.dma_start(bk_sb[:], bk_d[:])
        xt_sb = big.tile([P, 8, 2048], BF16)
        for o in range(8):  # per-o so the first proj chain starts ASAP
            nc.sync.dma_start(xt_sb[:, o, 0:512], xT4[0, :, o, :])
        wv_sb = wpool.tile([P, 8, 256], BF16)
        nc.sync.dma_start(wv_sb[:], wv_d[:])
        bv_sb = const.tile([1, 256], BF16)
        nc.sync.dma_start(bv_sb[:], bv_d[:])
        wq_sb = wpool.tile([P, 8, 256], BF16)
        nc.sync.dma_start(wq_sb[:], wq_d[:])
        bq_sb = const.tile([P, 2], F32)
        nc.sync.dma_start(bq_sb[:], bq_d[:])
        if causal:
            md_sb = const.tile([P, 256], BF16)
            nc.sync.dma_start(md_sb[:], md_d[:])
        id_sb = const.tile([P, P], BF16)
        nc.sync.dma_start(id_sb[:], id_d[:])
        wo_sb = wpool.tile([P, 2, 1024], BF16)
        nc.sync.dma_start(wo_sb[:], wo_d[:])
        for nch in range(1, 4):
            nc.sync.dma_start(xt_sb[:, :, 512 * nch:512 * nch + 512],
                              xT4[nch, :, :, :])

        ones_sb = const.tile([P, P], BF16)
        nc.gpsimd.memset(ones_sb[:], 1.0)
        qt_sb = big.tile([P, 2, 2048], BF16)    # [hd-of-pair, hp, tokens]
        kt_sb = big.tile([P, 2, 2048], BF16)
        va_sb = big.tile([P, 16, 260], BF16)    # [key, kb, 4x(64 v + 1 one)]
        ctxT_sb = big.tile([P, 2, 2048], BF16)  # [hd-of-pair, kt, tokens]
        nc.gpsimd.memset(
            va_sb.rearrange("p k (s c) -> p k s c", c=65)[:, :, :, 64:65], 1.0)

        # ---------- emission machinery ----------
        import collections
        fillers = collections.deque()
        lateq = collections.deque()     # deferrable work (output projection)
        pending = collections.deque()   # (qb, pair, et, first, last)

        def drain(n):
            for _ in range(n):
                if fillers:
                    fillers.popleft()()

        def drain_late(n):
            for _ in range(n):
                if lateq:
                    lateq.popleft()()

        # ---------- phase pieces ----------
        # projection chains run in the aux PSUM pool so that, when dispersed
        # between attention steps, they never starve the score-tile slots
        def proj_qk(w_sb, b_sb, dst_sb, nch, hp, pool, tag):
            pt = pool.tile([P, 512], F32, tag=tag, name=f"pp{nch}_{hp}")
            for kt in range(8):
                nc.tensor.matmul(
                    pt[:], w_sb[:, kt, 128 * hp:128 * hp + 128],
                    xt_sb[:, kt, 512 * nch:512 * nch + 512],
                    start=(kt == 0), stop=(kt == 7))
            nc.vector.tensor_scalar_add(
                dst_sb[:, hp, 512 * nch:512 * nch + 512], pt[:],
                b_sb[:, hp:hp + 1])

        def proj_v(tt, pool, tag):
            pt = pool.tile([P, 256], F32, tag=tag, name=f"pv{tt}")
            for kt in range(8):
                nc.tensor.matmul(
                    pt[:], xt_sb[:, kt, 128 * tt:128 * tt + 128],
                    wv_sb[:, kt, :], start=(kt == 0), stop=False)
            nc.tensor.matmul(pt[:], ones_sb[0:1, 0:P], bv_sb[:],
                             start=False, stop=True)
            nc.vector.tensor_copy(
                va_sb.rearrange("p k (s c) -> p k s c", c=65)[:, tt, :, 0:64],
                pt.rearrange("p (s c) -> p s c", c=64))

        def group_chains(nch, pool, tag):
            """All projection chains needed by q-block group `nch`."""
            ch = [lambda hp=hp: proj_qk(wk_sb, bk_sb, kt_sb, nch, hp,
                                        pool, tag) for hp in range(2)]
            ch += [lambda tt=tt: proj_v(tt, pool, tag)
                   for tt in range(4 * nch, 4 * nch + 4)]
            ch += [lambda hp=hp: proj_qk(wq_sb, bq_sb, qt_sb, nch, hp,
                                         pool, tag) for hp in range(2)]
            return ch

        projq = collections.deque()   # (group, chain-closure)

        def drain_proj(n):
            for _ in range(n):
                if projq:
                    projq.popleft()[1]()

        def force_proj(g):
            while projq and projq[0][0] <= g:
                projq.popleft()[1]()

        ctx_tiles = {}

        def emit_step(qb, pair):
            """Scores + exp (+ diag mask) for a kb-pair of one 128-q block."""
            sc = ps_sc.tile([P, 1024], F32, tag="sc",
                            name=f"sc{qb}_{pair[0]}")
            for u, kb in enumerate(pair):
                for s in range(4):
                    hb, hp = 64 * (s % 2), s // 2
                    nc.tensor.matmul(
                        sc[:, _col(u, s):_col(u, s) + 128],
                        kt_sb[hb:hb + 64, hp, 128 * kb:128 * kb + 128],
                        qt_sb[hb:hb + 64, hp, 128 * qb:128 * qb + 128],
                        start=True, stop=True)
            et = etp.tile([P, 1024], BF16, tag="et", name=f"et{qb}_{pair[0]}")
            if len(pair) == 2:
                nc.scalar.activation(et[:], sc[:], fexp)
            else:
                ap = sc.rearrange("p (b u c) -> p b u c", b=2, u=2)[:, :, 0, :]
                ep = et.rearrange("p (b u c) -> p b u c", b=2, u=2)[:, :, 0, :]
                nc.scalar.activation(ep, ap, fexp)
            if causal and pair[-1] == qb:
                u = len(pair) - 1
                nc.vector.tensor_tensor(
                    et[:, 256 * u:256 * u + 256],
                    et[:, 256 * u:256 * u + 256], md_sb[:], mult)
                nc.vector.tensor_tensor(
                    et[:, 512 + 256 * u:512 + 256 * u + 256],
                    et[:, 512 + 256 * u:512 + 256 * u + 256], md_sb[:], mult)
            pending.append((qb, pair, et))
            if len(pending) > LAG:
                emit_ctx(*pending.popleft())
            drain(K_FILL)
            drain_proj(1)
            if qb >= 10 and not projq:
                drain_late(3)

        def emit_ctx(qb, pair, et):
            nkb = qb + 1 if causal else 16
            if qb not in ctx_tiles:
                ctx_tiles[qb] = ps_ctx.tile([P, 260], F32, tag="ctx",
                                            name=f"ctx{qb}")
            cx = ctx_tiles[qb]
            for u, kb in enumerate(pair):
                for s in range(4):
                    nc.tensor.matmul(
                        cx[:, 65 * s:65 * s + 65],
                        et[:, _col(u, s):_col(u, s) + 128],
                        va_sb[:, kb, 65 * s:65 * s + 65],
                        start=(kb == 0 and s == 0),
                        stop=(kb == nkb - 1 and s == 3))
            if pair[-1] == nkb - 1:
                push_normalize(qb)

        def push_normalize(qb):
            cx = ctx_tiles[qb]
            cq = cqp.tile([P, 256], BF16, tag="cq", name=f"cq{qb}")
            recip = cqp.tile([P, 4], F32, tag="recip", name=f"rc{qb}")

            def f_recip():
                nc.vector.reciprocal(
                    recip[:],
                    cx.rearrange("p (s c) -> p s c", c=65)[:, :, 64])
            fillers.append(f_recip)
            for s in range(4):
                def f_mul(s=s):
                    nc.vector.tensor_scalar_mul(
                        cq[:, 64 * s:64 * s + 64],
                        cx[:, 65 * s:65 * s + 64], recip[:, s:s + 1])
                fillers.append(f_mul)
            for hp in range(2):
                def f_tr(hp=hp):
                    tr = ps_aux.tile([P, P], BF16, tag="aux",
                                     name=f"tr{qb}_{hp}")
                    nc.tensor.transpose(tr[:], cq[:, 128 * hp:128 * hp + 128],
                                        id_sb[:])
                    nc.vector.tensor_copy(
                        ctxT_sb[:, hp, 128 * qb:128 * qb + 128], tr[:])
                    if hp == 1 and qb % 2 == 1:
                        # ctxT for this q-block pair is now fully emitted ->
                        # its output projection may be scheduled (lateq)
                        push_oproj(qb // 2)
                fillers.append(f_tr)

        def push_oproj(grp):
            ob = obp.tile([P, 8, 256], F32, tag="ob", name=f"ob{grp}")
            for m in range(8):
                def f_mm(m=m):
                    po = ps_aux.tile([P, 256], F32, tag="aux",
                                     name=f"po{grp}_{m}")
                    for kt in range(2):
                        nc.tensor.matmul(
                            po[:], wo_sb[:, kt, 128 * m:128 * m + 128],
                            ctxT_sb[:, kt, 256 * grp:256 * grp + 256],
                            start=(kt == 0), stop=(kt == 1))
                    nc.vector.tensor_copy(ob[:, m, :], po[:])
                lateq.append(f_mm)

            def f_dma():
                nc.sync.dma_start(
                    outT.rearrange("(m p) t -> p m t", p=P)
                    [:, :, 256 * grp:256 * grp + 256], ob[:])
            lateq.append(f_dma)

        # ---------- schedule ----------
        def steps_of(qb):
            nkb = qb + 1 if causal else 16
            kbs = list(range(nkb))
            return [tuple(kbs[i:i + 2]) for i in range(0, nkb, 2)]

        if causal:
            # group 0's projections must run up front; later groups' chains
            # are dispersed between attention steps (aux pool) so the Act
            # engine's exp stream never starves while the PE does proj work
            for ch in group_chains(0, ps_sc, "sc"):
                ch()
                drain(1)
            for nch in range(1, 4):
                for ch in group_chains(nch, ps_aux, "aux"):
                    projq.append((nch, ch))
            for nch in range(4):
                force_proj(nch)
                for qb in range(4 * nch, 4 * nch + 4):
                    for pr in steps_of(qb):
                        emit_step(qb, pr)
        else:
            for nch in range(4):
                for ch in group_chains(nch, ps_sc, "sc"):
                    ch()
                    drain(1)
            for qb in range(16):
                for pr in steps_of(qb):
                    emit_step(qb, pr)

        while pending:
            emit_ctx(*pending.popleft())
        while fillers:
            fillers.popleft()()
        while lateq:
            lateq.popleft()()

    nc.compile()
    return nc


_CACHE = {}


def _get_nc(causal: bool):
    key = bool(causal)
    if key not in _CACHE:
        _CACHE[key] = _emit(key)
    return _CACHE[key]


def _mask():
    """Multiplicative causal mask for a diagonal 128x128 block, replicated
    across the 2 head slots that share a 256-col region."""
    i = np.arange(128)[:, None]
    j = np.arange(128)[None, :]
    m = (j >= i).astype(np.float32)
    return np.tile(m, (1, 2)).astype(NPBF)


def kernel(**inputs):
    x = np.asarray(inputs["x"], dtype=np.float32)
    Wq = np.asarray(inputs["Wq"], dtype=np.float32)
    bq = np.asarray(inputs["bq"], dtype=np.float32)
    Wk = np.asarray(inputs["Wk"], dtype=np.float32)
    bk = np.asarray(inputs["bk"], dtype=np.float32)
    Wv = np.asarray(inputs["Wv"], dtype=np.float32)
    bv = np.asarray(inputs["bv"], dtype=np.float32)
    Wo = np.asarray(inputs["Wo"], dtype=np.float32)
    bo = np.asarray(inputs["bo"], dtype=np.float32)
    causal = bool(int(np.asarray(inputs["enable_causal"])))

    scale = np.float32(1.0 / np.sqrt(HD))
    wqT = Wq.T                    # [in, out]
    wkT = (Wk * scale).T
    wvT = Wv.T
    woT = Wo.T                    # [ctx-dim, out]
    bks = bk * scale

    xs = []
    for b in range(B):
        xt = x[b].T.reshape(8, 128, 4, 512).transpose(2, 1, 0, 3)
        xs.append(np.ascontiguousarray(xt.astype(NPBF)))

    ident = np.eye(P, dtype=NPBF)
    if causal:
        md = _mask()

    nc = _get_nc(causal)
    in_maps = []
    for c in range(NC):
        b, g = divmod(c, 4)
        cols = slice(256 * g, 256 * g + 256)
        wq_r = np.ascontiguousarray(
            wqT[:, cols].reshape(8, 128, 256).transpose(1, 0, 2)).astype(NPBF)
        wk_r = np.ascontiguousarray(
            wkT[:, cols].reshape(8, 128, 256).transpose(1, 0, 2)).astype(NPBF)
        wv_r = np.ascontiguousarray(
            wvT[:, cols].reshape(8, 128, 256).transpose(1, 0, 2)).astype(NPBF)
        wo_r = np.ascontiguousarray(
            woT[cols, :].reshape(2, 128, 1024).transpose(1, 0, 2)).astype(NPBF)
        m = {"xT4": xs[b],
             "wq": wq_r, "wk": wk_r, "wv": wv_r, "wo": wo_r,
             "bq": np.ascontiguousarray(bq[cols].reshape(2, 128).T),
             "bk": np.ascontiguousarray(bks[cols].reshape(2, 128).T),
             "bv": np.ascontiguousarray(bv[cols].reshape(1, 256)).astype(NPBF),
             "ident": ident}
        if causal:
            m["md"] = md
        in_maps.append(m)

    global LAST_RESULT
    res = run_bass_kernel_spmd(nc, in_maps, list(range(NC)), trace=TRACE)
    LAST_RESULT = res

    # unshard: sum the 4 head-group partials per batch (row-parallel Wo), +bo
    out = np.empty((B, S, D), dtype=np.float32)
    for b in range(B):
        acc = res.results[4 * b]["outT"].astype(np.float32)
        for g in range(1, 4):
            acc = acc + res.results[4 * b + g]["outT"]
        out[b] = acc.T + bo[None, :]
    return out


# revision 36
# speedup vs baseline: 5.6284x; 1.0064x over previous
"""Trainium2 Bass kernel for nn_MultiHeadAttention (B=2, S=2048, D=1024, H=16, causal).

Sharding across 8 NeuronCores (single SPMD program, head-parallel TP):
  - Core c owns batch b=c//4 and head group g=c%4 (4 heads = 256 of the 1024
    projection columns).  W_q/W_k/W_v are column-sharded, W_o row-sharded.
  - Each core projects Q/K/V for ALL 2048 tokens of its batch but only its 4
    heads, runs full causal attention for those heads entirely in SBUF (no
    K/V exchange => ZERO collectives), then computes its partial output
    projection out_partial = ctx_heads @ Wo_rows.  The host unshard step sums
    the 4 partial outputs per batch and adds bo (the row-parallel reduction
    of tensor-parallel attention, folded into the host-side gather that the
    full-IO contract already requires).
  - bf16 operands everywhere on the PE (1 cycle/row at any N); f32 PSUM
    accumulation; 1/sqrt(64) folded into Wk/bk on the host.
  - Attention runs as a flat software-pipelined stream of (qb, kb-pair)
    steps at 128-query granularity: transposed scores scoresT[k, q] for all
    4 heads of two key-blocks land in one [128,1024] PSUM tile (parity-
    grouped so each PSUM bank only sees one PE tile row position - HW
    constraint), a single Act exp covers the pair, causal masking is
    multiplicative on the exp'd tile (diagonal blocks only), and the
    context matmuls are q-major (stationary = exp tile, moving = V plus a
    ones-column that yields the softmax denominator as column 64).  The
    denominator is then a per-partition scalar, so normalization is plain
    tensor_scalar multiplies; a PE transpose packs the normalized context
    back to hd-major for the output projection.
  - ctx matmuls lag the score stream (software pipelining) and the
    normalize/transpose/output-projection work is spread as filler between
    later steps, so PE, Act and DVE stay concurrently busy.
"""
import numpy as np
import ml_dtypes

import concourse.bass as bass
import concourse.bacc as bacc
import concourse.mybir as mybir
import concourse.tile as tile
from concourse.bass_utils import run_bass_kernel_spmd

B, S, D, H, HD = 2, 2048, 1024, 16, 64
NC = 8
P = 128
F32 = mybir.dt.float32
BF16 = mybir.dt.bfloat16
NPBF = ml_dtypes.bfloat16

TRACE = False        # set True (e.g. from test.py) to capture an NTFF profile
LAST_RESULT = None   # BassKernelResults of the most recent kernel() call

LAG = 4              # ctx stream lags the score stream by this many pair-steps
K_FILL = 3           # filler items drained per pair-step


def _col(u, s):
    """Column of head-slot s, pair-position u in the [128,1024] score tile.

    Parity-grouped: bank 0 (cols 0:512) holds even heads (PE row base 0),
    bank 1 (cols 512:1024) odd heads (row base 64) - matmuls into one PSUM
    bank must share a single PE tile row position.
    """
    return (0 if s % 2 == 0 else 512) + 256 * u + (128 if s >= 2 else 0)


def _emit(causal: bool):
    nc = bacc.Bacc(trn_type="TRN2", num_devices=NC)
    fexp = mybir.ActivationFunctionType.Exp
    mult = mybir.AluOpType.mult

    # ---- per-core DRAM inputs (host pre-sharded / pre-transposed) ----
    xT4 = nc.dram_tensor("xT4", [4, P, 8, 512], BF16, kind="ExternalInput")
    wq_d = nc.dram_tensor("wq", [P, 8, 256], BF16, kind="ExternalInput")
    wk_d = nc.dram_tensor("wk", [P, 8, 256], BF16, kind="ExternalInput")
    wv_d = nc.dram_tensor("wv", [P, 8, 256], BF16, kind="ExternalInput")
    wo_d = nc.dram_tensor("wo", [P, 2, 1024], BF16, kind="ExternalInput")
    bq_d = nc.dram_tensor("bq", [P, 2], F32, kind="ExternalInput")
    bk_d = nc.dram_tensor("bk", [P, 2], F32, kind="ExternalInput")
    bv_d = nc.dram_tensor("bv", [1, 256], BF16, kind="ExternalInput")
    id_d = nc.dram_tensor("ident", [P, P], BF16, kind="ExternalInput")
    if causal:
        md_d = nc.dram_tensor("md", [P, 256], BF16, kind="ExternalInput")
    outT = nc.dram_tensor("outT", [D, S], F32, kind="ExternalOutput")

    with tile.TileContext(nc) as tc, \
         tc.tile_pool(name="const", bufs=1) as const, \
         tc.tile_pool(name="w", bufs=1) as wpool, \
         tc.tile_pool(name="big", bufs=1) as big, \
         tc.tile_pool(name="et", bufs=5) as etp, \
         tc.tile_pool(name="cq", bufs=2) as cqp, \
         tc.tile_pool(name="ob", bufs=2) as obp, \
         tc.tile_pool(name="ps_sc", bufs=2, space="PSUM") as ps_sc, \
         tc.tile_pool(name="ps_ctx", bufs=2, space="PSUM") as ps_ctx, \
         tc.tile_pool(name="ps_aux", bufs=2, space="PSUM") as ps_aux:

        # ---------- constants / weights (critical-path DMA order) ----------
        wk_sb = wpool.tile([P, 8, 256], BF16)
        nc.sync.dma_start(wk_sb[:], wk_d[:])
        bk_sb = const.tile([P, 2], F32)
        nc.sync.dma_start(bk_sb[:], bk_d[:])
        xt_sb = big.tile([P, 8, 2048], BF16)
        for o in range(8):  # per-o so the first proj chain starts ASAP
            nc.sync.dma_start(xt_sb[:, o, 0:512], xT4[0, :, o, :])
        wv_sb = wpool.tile([P, 8, 256], BF16)
        nc.sync.dma_start(wv_sb[:], wv_d[:])
        bv_sb = const.tile([1, 256], BF16)
        nc.sync.dma_start(bv_sb[:], bv_d[:])
        wq_sb = wpool.tile([P, 8, 256], BF16)
        nc.sync.dma_start(wq_sb[:], wq_d[:])
        bq_sb = const.tile([P, 2], F32)
        nc.sync.dma_start(bq_sb[:], bq_d[:])
        if causal:
            md_sb = const.tile([P, 256], BF16)
            nc.sync.dma_start(md_sb[:], md_d[:])
        id_sb = const.tile([P, P], BF16)
        nc.sync.dma_start(id_sb[:], id_d[:])
        wo_sb = wpool.tile([P, 2, 1024], BF16)
        nc.sync.dma_start(wo_sb[:], wo_d[:])
        for nch in range(1, 4):
            nc.sync.dma_start(xt_sb[:, :, 512 * nch:512 * nch + 512],
                              xT4[nch, :, :, :])

        ones_sb = const.tile([P, P], BF16)
        nc.gpsimd.memset(ones_sb[:], 1.0)
        qt_sb = big.tile([P, 2, 2048], BF16)    # [hd-of-pair, hp, tokens]
        kt_sb = big.tile([P, 2, 2048], BF16)
        va_sb = big.tile([P, 16, 260], BF16)    # [key, kb, 4x(64 v + 1 one)]
        ctxT_sb = big.tile([P, 2, 2048], BF16)  # [hd-of-pair, kt, tokens]
        nc.gpsimd.memset(
            va_sb.rearrange("p k (s c) -> p k s c", c=65)[:, :, :, 64:65], 1.0)

        # ---------- emission machinery ----------
        import collections
        fillers = collections.deque()
        lateq = collections.deque()     # deferrable work (output projection)
        pending = collections.deque()   # (qb, pair, et, first, last)

        def drain(n):
            for _ in range(n):
                if fillers:
                    fillers.popleft()()

        def drain_late(n):
            for _ in range(n):
                if lateq:
                    lateq.popleft()()

        # ---------- phase pieces ----------
        # projection chains run in the aux PSUM pool so that, when dispersed
        # between attention steps, they never starve the score-tile slots
        def proj_qk(w_sb, b_sb, dst_sb, nch, hp, pool, tag):
            pt = pool.tile([P, 512], F32, tag=tag, name=f"pp{nch}_{hp}")
            for kt in range(8):
                nc.tensor.matmul(
                    pt[:], w_sb[:, kt, 128 * hp:128 * hp + 128],
                    xt_sb[:, kt, 512 * nch:512 * nch + 512],
                    start=(kt == 0), stop=(kt == 7))
            nc.vector.tensor_scalar_add(
                dst_sb[:, hp, 512 * nch:512 * nch + 512], pt[:],
                b_sb[:, hp:hp + 1])

        def proj_v(tt, pool, tag):
            pt = pool.tile([P, 256], F32, tag=tag, name=f"pv{tt}")
            for kt in range(8):
                nc.tensor.matmul(
                    pt[:], xt_sb[:, kt, 128 * tt:128 * tt + 128],
                    wv_sb[:, kt, :], start=(kt == 0), stop=False)
            nc.tensor.matmul(pt[:], ones_sb[0:1, 0:P], bv_sb[:],
                             start=False, stop=True)
            nc.vector.tensor_copy(
                va_sb.rearrange("p k (s c) -> p k s c", c=65)[:, tt, :, 0:64],
                pt.rearrange("p (s c) -> p s c", c=64))

        def group_chains(nch, pool, tag):
            """All projection chains needed by q-block group `nch`."""
            ch = [lambda hp=hp: proj_qk(wk_sb, bk_sb, kt_sb, nch, hp,
                                        pool, tag) for hp in range(2)]
            ch += [lambda tt=tt: proj_v(tt, pool, tag)
                   for tt in range(4 * nch, 4 * nch + 4)]
            ch += [lambda hp=hp: proj_qk(wq_sb, bq_sb, qt_sb, nch, hp,
                                         pool, tag) for hp in range(2)]
            return ch

        projq = collections.deque()   # (group, chain-closure)

        def drain_proj(n):
            for _ in range(n):
                if projq:
                    projq.popleft()[1]()

        def force_proj(g):
            while projq and projq[0][0] <= g:
                projq.popleft()[1]()

        ctx_tiles = {}

        def emit_step(qb, pair):
            """Scores + exp (+ diag mask) for a kb-pair of one 128-q block."""
            sc = ps_sc.tile([P, 1024], F32, tag="sc",
                            name=f"sc{qb}_{pair[0]}")
            for u, kb in enumerate(pair):
                for s in range(4):
                    hb, hp = 64 * (s % 2), s // 2
                    nc.tensor.matmul(
                        sc[:, _col(u, s):_col(u, s) + 128],
                        kt_sb[hb:hb + 64, hp, 128 * kb:128 * kb + 128],
                        qt_sb[hb:hb + 64, hp, 128 * qb:128 * qb + 128],
                        start=True, stop=True)
            et = etp.tile([P, 1024], BF16, tag="et", name=f"et{qb}_{pair[0]}")
            if len(pair) == 2:
                nc.scalar.activation(et[:], sc[:], fexp)
            else:
                ap = sc.rearrange("p (b u c) -> p b u c", b=2, u=2)[:, :, 0, :]
                ep = et.rearrange("p (b u c) -> p b u c", b=2, u=2)[:, :, 0, :]
                nc.scalar.activation(ep, ap, fexp)
            if causal and pair[-1] == qb:
                u = len(pair) - 1
                nc.vector.tensor_tensor(
                    et[:, 256 * u:256 * u + 256],
                    et[:, 256 * u:256 * u + 256], md_sb[:], mult)
                nc.vector.tensor_tensor(
                    et[:, 512 + 256 * u:512 + 256 * u + 256],
                    et[:, 512 + 256 * u:512 + 256 * u + 256], md_sb[:], mult)
            pending.append((qb, pair, et))
            if len(pending) > LAG:
                emit_ctx(*pending.popleft())
            drain(K_FILL)
            drain_proj(1)
            if qb >= 10 and not projq:
                drain_late(3)

        def emit_ctx(qb, pair, et):
            nkb = qb + 1 if causal else 16
            if qb not in ctx_tiles:
                ctx_tiles[qb] = ps_ctx.tile([P, 260], F32, tag="ctx",
                                            name=f"ctx{qb}")
            cx = ctx_tiles[qb]
            for u, kb in enumerate(pair):
                for s in range(4):
                    nc.tensor.matmul(
                        cx[:, 65 * s:65 * s + 65],
                        et[:, _col(u, s):_col(u, s) + 128],
                        va_sb[:, kb, 65 * s:65 * s + 65],
                        start=(kb == 0 and s == 0),
                        stop=(kb == nkb - 1 and s == 3))
            if pair[-1] == nkb - 1:
                push_normalize(qb)

        def push_normalize(qb):
            cx = ctx_tiles[qb]
            cq = cqp.tile([P, 256], BF16, tag="cq", name=f"cq{qb}")
            recip = cqp.tile([P, 4], F32, tag="recip", name=f"rc{qb}")

            def f_recip():
                nc.vector.reciprocal(
                    recip[:],
                    cx.rearrange("p (s c) -> p s c", c=65)[:, :, 64])
            fillers.append(f_recip)
            for s in range(4):
                def f_mul(s=s):
                    nc.vector.tensor_scalar_mul(
                        cq[:, 64 * s:64 * s + 64],
                        cx[:, 65 * s:65 * s + 64], recip[:, s:s + 1])
                fillers.append(f_mul)
            for hp in range(2):
                def f_tr(hp=hp):
                    tr = ps_aux.tile([P, P], BF16, tag="aux",
                                     name=f"tr{qb}_{hp}")
                    nc.tensor.transpose(tr[:], cq[:, 128 * hp:128 * hp + 128],
                                        id_sb[:])
                    nc.vector.tensor_copy(
                        ctxT_sb[:, hp, 128 * qb:128 * qb + 128], tr[:])
                    if hp == 1 and qb % 2 == 1:
                        # ctxT for this q-block pair is now fully emitted ->
                        # its output projection may be scheduled (lateq)
                        push_oproj(qb // 2)
                fillers.append(f_tr)

        def push_oproj(grp):
            ob = obp.tile([P, 8, 256], F32, tag="ob", name=f"ob{grp}")
            for m in range(8):
                def f_mm(m=m):
                    po = ps_aux.tile([P, 256], F32, tag="aux",
                                     name=f"po{grp}_{m}")
                    for kt in range(2):
                        nc.tensor.matmul(
                            po[:], wo_sb[:, kt, 128 * m:128 * m + 128],
                            ctxT_sb[:, kt, 256 * grp:256 * grp + 256],
                            start=(kt == 0), stop=(kt == 1))
                    nc.vector.tensor_copy(ob[:, m, :], po[:])
                lateq.append(f_mm)

            def f_dma():
                nc.sync.dma_start(
                    outT.rearrange("(m p) t -> p m t", p=P)
                    [:, :, 256 * grp:256 * grp + 256], ob[:])
            lateq.append(f_dma)

        # ---------- schedule ----------
        def steps_of(qb):
            nkb = qb + 1 if causal else 16
            kbs = list(range(nkb))
            return [tuple(kbs[i:i + 2]) for i in range(0, nkb, 2)]

        if causal:
            # group 0's projections must run up front; later groups' chains
            # are dispersed between attention steps (aux pool) so the Act
            # engine's exp stream never starves while the PE does proj work
            for ch in group_chains(0, ps_sc, "sc"):
                ch()
                drain(1)
            for nch in range(1, 4):
                for ch in group_chains(nch, ps_aux, "aux"):
                    projq.append((nch, ch))
            for nch in range(4):
                force_proj(nch)
                for qb in range(4 * nch, 4 * nch + 4):
                    for pr in steps_of(qb):
                        emit_step(qb, pr)
        else:
            for nch in range(4):
                for ch in group_chains(nch, ps_sc, "sc"):
                    ch()
                    drain(1)
            for qb in range(16):
                for pr in steps_of(qb):
                    emit_step(qb, pr)

        while pending:
            emit_ctx(*pending.popleft())
        while fillers:
            fillers.popleft()()
        while lateq:
            lateq.popleft()()

    nc.compile()
    return nc


_CACHE = {}


def _get_nc(causal: bool):
    key = bool(causal)
    if key not in _CACHE:
        _CACHE[key] = _emit(key)
    return _CACHE[key]


def _mask():
    """Multiplicative causal mask for a diagonal 128x128 block, replicated
    across the 2 head slots that share a 256-col region."""
    i = np.arange(128)[:, None]
    j = np.arange(128)[None, :]
    m = (j >= i).astype(np.float32)
    return np.tile(m, (1, 2)).astype(NPBF)


def kernel(**inputs):
    x = np.asarray(inputs["x"], dtype=np.float32)
    Wq = np.asarray(inputs["Wq"], dtype=np.float32)
    bq = np.asarray(inputs["bq"], dtype=np.float32)
    Wk = np.asarray(inputs["Wk"], dtype=np.float32)
    bk = np.asarray(inputs["bk"], dtype=np.float32)
    Wv = np.asarray(inputs["Wv"], dtype=np.float32)
    bv = np.asarray(inputs["bv"], dtype=np.float32)
    Wo = np.asarray(inputs["Wo"], dtype=np.float32)
    bo = np.asarray(inputs["bo"], dtype=np.float32)
    causal = bool(int(np.asarray(inputs["enable_causal"])))

    scale = np.float32(1.0 / np.sqrt(HD))
    wqT = Wq.T                    # [in, out]
    wkT = (Wk * scale).T
    wvT = Wv.T
    woT = Wo.T                    # [ctx-dim, out]
    bks = bk * scale

    xs = []
    for b in range(B):
        xt = x[b].T.reshape(8, 128, 4, 512).transpose(2, 1, 0, 3)
        xs.append(np.ascontiguousarray(xt.astype(NPBF)))

    ident = np.eye(P, dtype=NPBF)
    if causal:
        md = _mask()

    nc = _get_nc(causal)
    in_maps = []
    for c in range(NC):
        b, g = divmod(c, 4)
        cols = slice(256 * g, 256 * g + 256)
        wq_r = np.ascontiguousarray(
            wqT[:, cols].reshape(8, 128, 256).transpose(1, 0, 2)).astype(NPBF)
        wk_r = np.ascontiguousarray(
            wkT[:, cols].reshape(8, 128, 256).transpose(1, 0, 2)).astype(NPBF)
        wv_r = np.ascontiguousarray(
            wvT[:, cols].reshape(8, 128, 256).transpose(1, 0, 2)).astype(NPBF)
        wo_r = np.ascontiguousarray(
            woT[cols, :].reshape(2, 128, 1024).transpose(1, 0, 2)).astype(NPBF)
        m = {"xT4": xs[b],
             "wq": wq_r, "wk": wk_r, "wv": wv_r, "wo": wo_r,
             "bq": np.ascontiguousarray(bq[cols].reshape(2, 128).T),
             "bk": np.ascontiguousarray(bks[cols].reshape(2, 128).T),
             "bv": np.ascontiguousarray(bv[cols].reshape(1, 256)).astype(NPBF),
             "ident": ident}
        if causal:
            m["md"] = md
        in_maps.append(m)

    global LAST_RESULT
    res = run_bass_kernel_spmd(nc, in_maps, list(range(NC)), trace=TRACE)
    LAST_RESULT = res

    # unshard: sum the 4 head-group partials per batch (row-parallel Wo), +bo
    out = np.empty((B, S, D), dtype=np.float32)
    for b in range(B):
        acc = res.results[4 * b]["outT"].astype(np.float32)
        for g in range(1, 4):
            acc = acc + res.results[4 * b + g]["outT"]
        out[b] = acc.T + bo[None, :]
    return out


# revision 42
# speedup vs baseline: 5.7009x; 1.0129x over previous
"""Trainium2 Bass kernel for nn_MultiHeadAttention (B=2, S=2048, D=1024, H=16, causal).

Sharding across 8 NeuronCores (single SPMD program, head-parallel TP):
  - Core c owns batch b=c//4 and head group g=c%4 (4 heads = 256 of the 1024
    projection columns).  W_q/W_k/W_v are column-sharded, W_o row-sharded.
  - Each core projects Q/K/V for ALL 2048 tokens of its batch but only its 4
    heads, runs full causal attention for those heads entirely in SBUF (no
    K/V exchange => ZERO collectives), then computes its partial output
    projection out_partial = ctx_heads @ Wo_rows.  The host unshard step sums
    the 4 partial outputs per batch and adds bo (the row-parallel reduction
    of tensor-parallel attention, folded into the host-side gather that the
    full-IO contract already requires).
  - bf16 operands everywhere on the PE (1 cycle/row at any N); f32 PSUM
    accumulation; 1/sqrt(64) folded into Wk/bk on the host.
  - Attention runs as a flat software-pipelined stream of (qb, kb-pair)
    steps at 128-query granularity: transposed scores scoresT[k, q] for all
    4 heads of two key-blocks land in one [128,1024] PSUM tile (parity-
    grouped so each PSUM bank only sees one PE tile row position - HW
    constraint), a single Act exp covers the pair, causal masking is
    multiplicative on the exp'd tile (diagonal blocks only), and the
    context matmuls are q-major (stationary = exp tile, moving = V plus a
    ones-column that yields the softmax denominator as column 64).  The
    denominator is then a per-partition scalar, so normalization is plain
    tensor_scalar multiplies; a PE transpose packs the normalized context
    back to hd-major for the output projection.
  - ctx matmuls lag the score stream (software pipelining) and the
    normalize/transpose/output-projection work is spread as filler between
    later steps, so PE, Act and DVE stay concurrently busy.
"""
import numpy as np
import ml_dtypes

import concourse.bass as bass
import concourse.bacc as bacc
import concourse.mybir as mybir
import concourse.tile as tile
from concourse.bass_utils import run_bass_kernel_spmd

B, S, D, H, HD = 2, 2048, 1024, 16, 64
NC = 8
P = 128
F32 = mybir.dt.float32
BF16 = mybir.dt.bfloat16
NPBF = ml_dtypes.bfloat16

TRACE = False        # set True (e.g. from test.py) to capture an NTFF profile
LAST_RESULT = None   # BassKernelResults of the most recent kernel() call

LAG = 4              # ctx stream lags the score stream by this many pair-steps
K_FILL = 3           # filler items drained per pair-step


def _col(u, s):
    """Column of head-slot s, pair-position u in the [128,1024] score tile.

    Parity-grouped: bank 0 (cols 0:512) holds even heads (PE row base 0),
    bank 1 (cols 512:1024) odd heads (row base 64) - matmuls into one PSUM
    bank must share a single PE tile row position.
    """
    return (0 if s % 2 == 0 else 512) + 256 * u + (128 if s >= 2 else 0)


def _emit(causal: bool):
    nc = bacc.Bacc(trn_type="TRN2", num_devices=NC)
    fexp = mybir.ActivationFunctionType.Exp
    mult = mybir.AluOpType.mult

    # ---- per-core DRAM inputs (host pre-sharded / pre-transposed) ----
    xT4 = nc.dram_tensor("xT4", [4, P, 8, 512], BF16, kind="ExternalInput")
    wq_d = nc.dram_tensor("wq", [P, 8, 256], BF16, kind="ExternalInput")
    wk_d = nc.dram_tensor("wk", [P, 8, 256], BF16, kind="ExternalInput")
    wv_d = nc.dram_tensor("wv", [P, 8, 256], BF16, kind="ExternalInput")
    wo_d = nc.dram_tensor("wo", [P, 2, 1024], BF16, kind="ExternalInput")
    bq_d = nc.dram_tensor("bq", [P, 2], F32, kind="ExternalInput")
    bk_d = nc.dram_tensor("bk", [P, 2], F32, kind="ExternalInput")
    bv_d = nc.dram_tensor("bv", [1, 256], BF16, kind="ExternalInput")
    id_d = nc.dram_tensor("ident", [P, P], BF16, kind="ExternalInput")
    if causal:
        md_d = nc.dram_tensor("md", [P, 256], BF16, kind="ExternalInput")
    outT = nc.dram_tensor("outT", [D, S], F32, kind="ExternalOutput")

    with tile.TileContext(nc) as tc, \
         tc.tile_pool(name="const", bufs=1) as const, \
         tc.tile_pool(name="w", bufs=1) as wpool, \
         tc.tile_pool(name="big", bufs=1) as big, \
         tc.tile_pool(name="et", bufs=5) as etp, \
         tc.tile_pool(name="cq", bufs=2) as cqp, \
         tc.tile_pool(name="ob", bufs=2) as obp, \
         tc.tile_pool(name="ps_sc", bufs=2, space="PSUM") as ps_sc, \
         tc.tile_pool(name="ps_ctx", bufs=2, space="PSUM") as ps_ctx, \
         tc.tile_pool(name="ps_aux", bufs=2, space="PSUM") as ps_aux:

        # ---------- constants / weights (critical-path DMA order) ----------
        wk_sb = wpool.tile([P, 8, 256], BF16)
        nc.sync.dma_start(wk_sb[:], wk_d[:])
        bk_sb = const.tile([P, 2], F32)
        nc.sync.dma_start(bk_sb[:], bk_d[:])
        xt_sb = big.tile([P, 8, 2048], BF16)
        for o in range(8):  # per-o so the first proj chain starts ASAP
            nc.sync.dma_start(xt_sb[:, o, 0:512], xT4[0, :, o, :])
        wv_sb = wpool.tile([P, 8, 256], BF16)
        nc.sync.dma_start(wv_sb[:], wv_d[:])
        bv_sb = const.tile([1, 256], BF16)
        nc.sync.dma_start(bv_sb[:], bv_d[:])
        wq_sb = wpool.tile([P, 8, 256], BF16)
        nc.sync.dma_start(wq_sb[:], wq_d[:])
        bq_sb = const.tile([P, 2], F32)
        nc.sync.dma_start(bq_sb[:], bq_d[:])
        if causal:
            md_sb = const.tile([P, 256], BF16)
            nc.sync.dma_start(md_sb[:], md_d[:])
        id_sb = const.tile([P, P], BF16)
        nc.sync.dma_start(id_sb[:], id_d[:])
        wo_sb = wpool.tile([P, 2, 1024], BF16)
        nc.sync.dma_start(wo_sb[:], wo_d[:])
        for nch in range(1, 4):
            nc.sync.dma_start(xt_sb[:, :, 512 * nch:512 * nch + 512],
                              xT4[nch, :, :, :])

        ones_sb = const.tile([P, P], BF16)
        nc.gpsimd.memset(ones_sb[:], 1.0)
        # bv broadcast to all partitions once; folded into the V copy as a
        # DVE add instead of 16 per-tile bias matmuls
        bb_ps = ps_aux.tile([P, 256], F32, tag="aux", name="bb")
        nc.tensor.matmul(bb_ps[:], ones_sb[0:1, 0:P], bv_sb[:],
                         start=True, stop=True)
        bvb_sb = const.tile([P, 256], BF16)
        nc.vector.tensor_copy(bvb_sb[:], bb_ps[:])
        qt_sb = big.tile([P, 2, 2048], BF16)    # [hd-of-pair, hp, tokens]
        kt_sb = big.tile([P, 2, 2048], BF16)
        va_sb = big.tile([P, 16, 260], BF16)    # [key, kb, 4x(64 v + 1 one)]
        ctxT_sb = big.tile([P, 2, 2048], BF16)  # [hd-of-pair, kt, tokens]
        nc.gpsimd.memset(
            va_sb.rearrange("p k (s c) -> p k s c", c=65)[:, :, :, 64:65], 1.0)

        # ---------- emission machinery ----------
        import collections
        fillers = collections.deque()
        lateq = collections.deque()     # deferrable work (output projection)
        pending = collections.deque()   # (qb, pair, et, first, last)

        def drain(n):
            for _ in range(n):
                if fillers:
                    fillers.popleft()()

        def drain_late(n):
            for _ in range(n):
                if lateq:
                    lateq.popleft()()

        # ---------- phase pieces ----------
        # projection chains run in the aux PSUM pool so that, when dispersed
        # between attention steps, they never starve the score-tile slots
        def proj_qk(w_sb, b_sb, dst_sb, nch, hp, pool, tag):
            pt = pool.tile([P, 512], F32, tag=tag, name=f"pp{nch}_{hp}")
            for kt in range(8):
                nc.tensor.matmul(
                    pt[:], w_sb[:, kt, 128 * hp:128 * hp + 128],
                    xt_sb[:, kt, 512 * nch:512 * nch + 512],
                    start=(kt == 0), stop=(kt == 7))
            nc.vector.tensor_scalar_add(
                dst_sb[:, hp, 512 * nch:512 * nch + 512], pt[:],
                b_sb[:, hp:hp + 1])

        def proj_v(tt, pool, tag):
            pt = pool.tile([P, 256], F32, tag=tag, name=f"pv{tt}")
            for kt in range(8):
                nc.tensor.matmul(
                    pt[:], xt_sb[:, kt, 128 * tt:128 * tt + 128],
                    wv_sb[:, kt, :], start=(kt == 0), stop=(kt == 7))
            nc.vector.tensor_tensor(
                va_sb.rearrange("p k (s c) -> p k s c", c=65)[:, tt, :, 0:64],
                pt.rearrange("p (s c) -> p s c", c=64),
                bvb_sb.rearrange("p (s c) -> p s c", c=64),
                mybir.AluOpType.add)

        def group_chains(nch, pool, tag):
            """All projection chains needed by q-block group `nch`."""
            ch = [lambda hp=hp: proj_qk(wk_sb, bk_sb, kt_sb, nch, hp,
                                        pool, tag) for hp in range(2)]
            ch += [lambda tt=tt: proj_v(tt, pool, tag)
                   for tt in range(4 * nch, 4 * nch + 4)]
            ch += [lambda hp=hp: proj_qk(wq_sb, bq_sb, qt_sb, nch, hp,
                                         pool, tag) for hp in range(2)]
            return ch

        projq = collections.deque()   # (group, chain-closure)

        def drain_proj(n):
            for _ in range(n):
                if projq:
                    projq.popleft()[1]()

        def force_proj(g):
            while projq and projq[0][0] <= g:
                projq.popleft()[1]()

        ctx_tiles = {}

        def emit_step(qb, pair):
            """Scores + exp (+ diag mask) for a kb-pair of one 128-q block."""
            sc = ps_sc.tile([P, 1024], F32, tag="sc",
                            name=f"sc{qb}_{pair[0]}")
            for u, kb in enumerate(pair):
                for s in range(4):
                    hb, hp = 64 * (s % 2), s // 2
                    nc.tensor.matmul(
                        sc[:, _col(u, s):_col(u, s) + 128],
                        kt_sb[hb:hb + 64, hp, 128 * kb:128 * kb + 128],
                        qt_sb[hb:hb + 64, hp, 128 * qb:128 * qb + 128],
                        start=True, stop=True)
            et = etp.tile([P, 1024], BF16, tag="et", name=f"et{qb}_{pair[0]}")
            if len(pair) == 2:
                nc.scalar.activation(et[:], sc[:], fexp)
            else:
                ap = sc.rearrange("p (b u c) -> p b u c", b=2, u=2)[:, :, 0, :]
                ep = et.rearrange("p (b u c) -> p b u c", b=2, u=2)[:, :, 0, :]
                nc.scalar.activation(ep, ap, fexp)
            if causal and pair[-1] == qb:
                u = len(pair) - 1
                nc.vector.tensor_tensor(
                    et[:, 256 * u:256 * u + 256],
                    et[:, 256 * u:256 * u + 256], md_sb[:], mult)
                nc.vector.tensor_tensor(
                    et[:, 512 + 256 * u:512 + 256 * u + 256],
                    et[:, 512 + 256 * u:512 + 256 * u + 256], md_sb[:], mult)
            pending.append((qb, pair, et))
            if len(pending) > LAG:
                emit_ctx(*pending.popleft())
            drain(K_FILL)
            drain_proj(1)
            if qb >= 10 and not projq:
                drain_late(3)

        def emit_ctx(qb, pair, et):
            nkb = qb + 1 if causal else 16
            if qb not in ctx_tiles:
                ctx_tiles[qb] = ps_ctx.tile([P, 260], F32, tag="ctx",
                                            name=f"ctx{qb}")
            cx = ctx_tiles[qb]
            for u, kb in enumerate(pair):
                for s in range(4):
                    nc.tensor.matmul(
                        cx[:, 65 * s:65 * s + 65],
                        et[:, _col(u, s):_col(u, s) + 128],
                        va_sb[:, kb, 65 * s:65 * s + 65],
                        start=(kb == 0 and s == 0),
                        stop=(kb == nkb - 1 and s == 3))
            if pair[-1] == nkb - 1:
                push_normalize(qb)

        def push_normalize(qb):
            cx = ctx_tiles[qb]
            cq = cqp.tile([P, 256], BF16, tag="cq", name=f"cq{qb}")
            recip = cqp.tile([P, 4], F32, tag="recip", name=f"rc{qb}")

            def f_recip():
                nc.vector.reciprocal(
                    recip[:],
                    cx.rearrange("p (s c) -> p s c", c=65)[:, :, 64])
            fillers.append(f_recip)
            for s in range(4):
                def f_mul(s=s):
                    nc.vector.tensor_scalar_mul(
                        cq[:, 64 * s:64 * s + 64],
                        cx[:, 65 * s:65 * s + 64], recip[:, s:s + 1])
                fillers.append(f_mul)
            for hp in range(2):
                def f_tr(hp=hp):
                    tr = ps_aux.tile([P, P], BF16, tag="aux",
                                     name=f"tr{qb}_{hp}")
                    nc.tensor.transpose(tr[:], cq[:, 128 * hp:128 * hp + 128],
                                        id_sb[:])
                    nc.vector.tensor_copy(
                        ctxT_sb[:, hp, 128 * qb:128 * qb + 128], tr[:])
                    if hp == 1 and qb % 2 == 1:
                        # ctxT for this q-block pair is now fully emitted ->
                        # its output projection may be scheduled (lateq)
                        push_oproj(qb // 2)
                fillers.append(f_tr)

        def push_oproj(grp):
            ob = obp.tile([P, 8, 256], F32, tag="ob", name=f"ob{grp}")
            for m in range(8):
                def f_mm(m=m):
                    po = ps_aux.tile([P, 256], F32, tag="aux",
                                     name=f"po{grp}_{m}")
                    for kt in range(2):
                        nc.tensor.matmul(
                            po[:], wo_sb[:, kt, 128 * m:128 * m + 128],
                            ctxT_sb[:, kt, 256 * grp:256 * grp + 256],
                            start=(kt == 0), stop=(kt == 1))
                    nc.vector.tensor_copy(ob[:, m, :], po[:])
                lateq.append(f_mm)

            def f_dma():
                nc.sync.dma_start(
                    outT.rearrange("(m p) t -> p m t", p=P)
                    [:, :, 256 * grp:256 * grp + 256], ob[:])
            lateq.append(f_dma)

        # ---------- schedule ----------
        def steps_of(qb):
            nkb = qb + 1 if causal else 16
            kbs = list(range(nkb))
            return [tuple(kbs[i:i + 2]) for i in range(0, nkb, 2)]

        if causal:
            # group 0's projections must run up front; later groups' chains
            # are dispersed between attention steps (aux pool) so the Act
            # engine's exp stream never starves while the PE does proj work
            for ch in group_chains(0, ps_sc, "sc"):
                ch()
                drain(1)
            for nch in range(1, 4):
                for ch in group_chains(nch, ps_aux, "aux"):
                    projq.append((nch, ch))
            for nch in range(4):
                force_proj(nch)
                for qb in range(4 * nch, 4 * nch + 4):
                    for pr in steps_of(qb):
                        emit_step(qb, pr)
        else:
            for nch in range(4):
                for ch in group_chains(nch, ps_sc, "sc"):
                    ch()
                    drain(1)
            for qb in range(16):
                for pr in steps_of(qb):
                    emit_step(qb, pr)

        while pending:
            emit_ctx(*pending.popleft())
        while fillers:
            fillers.popleft()()
        while lateq:
            lateq.popleft()()

    nc.compile()
    return nc


_CACHE = {}


def _get_nc(causal: bool):
    key = bool(causal)
    if key not in _CACHE:
        _CACHE[key] = _emit(key)
    return _CACHE[key]


def _mask():
    """Multiplicative causal mask for a diagonal 128x128 block, replicated
    across the 2 head slots that share a 256-col region."""
    i = np.arange(128)[:, None]
    j = np.arange(128)[None, :]
    m = (j >= i).astype(np.float32)
    return np.tile(m, (1, 2)).astype(NPBF)


def kernel(**inputs):
    x = np.asarray(inputs["x"], dtype=np.float32)
    Wq = np.asarray(inputs["Wq"], dtype=np.float32)
    bq = np.asarray(inputs["bq"], dtype=np.float32)
    Wk = np.asarray(inputs["Wk"], dtype=np.float32)
    bk = np.asarray(inputs["bk"], dtype=np.float32)
    Wv = np.asarray(inputs["Wv"], dtype=np.float32)
    bv = np.asarray(inputs["bv"], dtype=np.float32)
    Wo = np.asarray(inputs["Wo"], dtype=np.float32)
    bo = np.asarray(inputs["bo"], dtype=np.float32)
    causal = bool(int(np.asarray(inputs["enable_causal"])))

    scale = np.float32(1.0 / np.sqrt(HD))
    wqT = Wq.T                    # [in, out]
    wkT = (Wk * scale).T
    wvT = Wv.T
    woT = Wo.T                    # [ctx-dim, out]
    bks = bk * scale

    xs = []
    for b in range(B):
        xt = x[b].T.reshape(8, 128, 4, 512).transpose(2, 1, 0, 3)
        xs.append(np.ascontiguousarray(xt.astype(NPBF)))

    ident = np.eye(P, dtype=NPBF)
    if causal:
        md = _mask()

    nc = _get_nc(causal)
    in_maps = []
    for c in range(NC):
        b, g = divmod(c, 4)
        cols = slice(256 * g, 256 * g + 256)
        wq_r = np.ascontiguousarray(
            wqT[:, cols].reshape(8, 128, 256).transpose(1, 0, 2)).astype(NPBF)
        wk_r = np.ascontiguousarray(
            wkT[:, cols].reshape(8, 128, 256).transpose(1, 0, 2)).astype(NPBF)
        wv_r = np.ascontiguousarray(
            wvT[:, cols].reshape(8, 128, 256).transpose(1, 0, 2)).astype(NPBF)
        wo_r = np.ascontiguousarray(
            woT[cols, :].reshape(2, 128, 1024).transpose(1, 0, 2)).astype(NPBF)
        m = {"xT4": xs[b],
             "wq": wq_r, "wk": wk_r, "wv": wv_r, "wo": wo_r,
             "bq": np.ascontiguousarray(bq[cols].reshape(2, 128).T),
             "bk": np.ascontiguousarray(bks[cols].reshape(2, 128).T),
             "bv": np.ascontiguousarray(bv[cols].reshape(1, 256)).astype(NPBF),
             "ident": ident}
        if causal:
            m["md"] = md
        in_maps.append(m)

    global LAST_RESULT
    res = run_bass_kernel_spmd(nc, in_maps, list(range(NC)), trace=TRACE)
    LAST_RESULT = res

    # unshard: sum the 4 head-group partials per batch (row-parallel Wo), +bo
    out = np.empty((B, S, D), dtype=np.float32)
    for b in range(B):
        acc = res.results[4 * b]["outT"].astype(np.float32)
        for g in range(1, 4):
            acc = acc + res.results[4 * b + g]["outT"]
        out[b] = acc.T + bo[None, :]
    return out


# revision 43
# speedup vs baseline: 5.7974x; 1.0169x over previous
"""Trainium2 Bass kernel for nn_MultiHeadAttention (B=2, S=2048, D=1024, H=16, causal).

Sharding across 8 NeuronCores (single SPMD program, head-parallel TP):
  - Core c owns batch b=c//4 and head group g=c%4 (4 heads = 256 of the 1024
    projection columns).  W_q/W_k/W_v are column-sharded, W_o row-sharded.
  - Each core projects Q/K/V for ALL 2048 tokens of its batch but only its 4
    heads, runs full causal attention for those heads entirely in SBUF (no
    K/V exchange => ZERO collectives), then computes its partial output
    projection out_partial = ctx_heads @ Wo_rows.  The host unshard step sums
    the 4 partial outputs per batch and adds bo (the row-parallel reduction
    of tensor-parallel attention, folded into the host-side gather that the
    full-IO contract already requires).
  - bf16 operands everywhere on the PE (1 cycle/row at any N); f32 PSUM
    accumulation; 1/sqrt(64) folded into Wk/bk on the host.
  - Attention runs as a flat software-pipelined stream of (qb, kb-pair)
    steps at 128-query granularity: transposed scores scoresT[k, q] for all
    4 heads of two key-blocks land in one [128,1024] PSUM tile (parity-
    grouped so each PSUM bank only sees one PE tile row position - HW
    constraint), a single Act exp covers the pair, causal masking is
    multiplicative on the exp'd tile (diagonal blocks only), and the
    context matmuls are q-major (stationary = exp tile, moving = V plus a
    ones-column that yields the softmax denominator as column 64).  The
    denominator is then a per-partition scalar, so normalization is plain
    tensor_scalar multiplies; a PE transpose packs the normalized context
    back to hd-major for the output projection.
  - ctx matmuls lag the score stream (software pipelining) and the
    normalize/transpose/output-projection work is spread as filler between
    later steps, so PE, Act and DVE stay concurrently busy.
"""
import numpy as np
import ml_dtypes

import concourse.bass as bass
import concourse.bacc as bacc
import concourse.mybir as mybir
import concourse.tile as tile
from concourse.bass_utils import run_bass_kernel_spmd

B, S, D, H, HD = 2, 2048, 1024, 16, 64
NC = 8
P = 128
F32 = mybir.dt.float32
BF16 = mybir.dt.bfloat16
NPBF = ml_dtypes.bfloat16

TRACE = False        # set True (e.g. from test.py) to capture an NTFF profile
LAST_RESULT = None   # BassKernelResults of the most recent kernel() call

LAG = 4              # ctx stream lags the score stream by this many pair-steps
K_FILL = 3           # filler items drained per pair-step


def _col(u, s):
    """Column of head-slot s, pair-position u in the [128,1024] score tile.

    Parity-grouped: bank 0 (cols 0:512) holds even heads (PE row base 0),
    bank 1 (cols 512:1024) odd heads (row base 64) - matmuls into one PSUM
    bank must share a single PE tile row position.
    """
    return (0 if s % 2 == 0 else 512) + 256 * u + (128 if s >= 2 else 0)


def _emit(causal: bool):
    nc = bacc.Bacc(trn_type="TRN2", num_devices=NC)
    fexp = mybir.ActivationFunctionType.Exp
    mult = mybir.AluOpType.mult

    # ---- per-core DRAM inputs (host pre-sharded / pre-transposed) ----
    xT4 = nc.dram_tensor("xT4", [4, P, 8, 512], BF16, kind="ExternalInput")
    wq_d = nc.dram_tensor("wq", [P, 8, 256], BF16, kind="ExternalInput")
    wk_d = nc.dram_tensor("wk", [P, 8, 256], BF16, kind="ExternalInput")
    wv_d = nc.dram_tensor("wv", [P, 8, 256], BF16, kind="ExternalInput")
    wo_d = nc.dram_tensor("wo", [P, 2, 1024], BF16, kind="ExternalInput")
    bq_d = nc.dram_tensor("bq", [P, 2], F32, kind="ExternalInput")
    bk_d = nc.dram_tensor("bk", [P, 2], F32, kind="ExternalInput")
    bv_d = nc.dram_tensor("bv", [1, 256], BF16, kind="ExternalInput")
    id_d = nc.dram_tensor("ident", [P, P], BF16, kind="ExternalInput")
    if causal:
        md_d = nc.dram_tensor("md", [P, 256], BF16, kind="ExternalInput")
    outT = nc.dram_tensor("outT", [D, S], F32, kind="ExternalOutput")

    with tile.TileContext(nc) as tc, \
         tc.tile_pool(name="const", bufs=1) as const, \
         tc.tile_pool(name="w", bufs=1) as wpool, \
         tc.tile_pool(name="big", bufs=1) as big, \
         tc.tile_pool(name="et", bufs=5) as etp, \
         tc.tile_pool(name="cq", bufs=2) as cqp, \
         tc.tile_pool(name="ob", bufs=2) as obp, \
         tc.tile_pool(name="ps_sc", bufs=2, space="PSUM") as ps_sc, \
         tc.tile_pool(name="ps_ctx", bufs=2, space="PSUM") as ps_ctx, \
         tc.tile_pool(name="ps_aux", bufs=2, space="PSUM") as ps_aux:

        # ---------- constants / weights (critical-path DMA order) ----------
        wk_sb = wpool.tile([P, 8, 256], BF16)
        nc.sync.dma_start(wk_sb[:], wk_d[:])
        bk_sb = const.tile([P, 2], F32)
        nc.sync.dma_start(bk_sb[:], bk_d[:])
        xt_sb = big.tile([P, 8, 2048], BF16)
        for o in range(8):  # per-o so the first proj chain starts ASAP
            nc.sync.dma_start(xt_sb[:, o, 0:512], xT4[0, :, o, :])
        wv_sb = wpool.tile([P, 8, 256], BF16)
        nc.sync.dma_start(wv_sb[:], wv_d[:])
        bv_sb = const.tile([1, 256], BF16)
        nc.sync.dma_start(bv_sb[:], bv_d[:])
        wq_sb = wpool.tile([P, 8, 256], BF16)
        nc.sync.dma_start(wq_sb[:], wq_d[:])
        bq_sb = const.tile([P, 2], F32)
        nc.sync.dma_start(bq_sb[:], bq_d[:])
        if causal:
            md_sb = const.tile([P, 256], BF16)
            nc.sync.dma_start(md_sb[:], md_d[:])
        id_sb = const.tile([P, P], BF16)
        nc.sync.dma_start(id_sb[:], id_d[:])
        wo_sb = wpool.tile([P, 2, 1024], BF16)
        nc.sync.dma_start(wo_sb[:], wo_d[:])
        for nch in range(1, 4):
            nc.sync.dma_start(xt_sb[:, :, 512 * nch:512 * nch + 512],
                              xT4[nch, :, :, :])

        ones_sb = const.tile([P, P], BF16)
        nc.gpsimd.memset(ones_sb[:], 1.0)
        # bv broadcast to all partitions once; folded into the V copy as a
        # DVE add instead of 16 per-tile bias matmuls
        bb_ps = ps_aux.tile([P, 256], F32, tag="aux", name="bb")
        nc.tensor.matmul(bb_ps[:], ones_sb[0:1, 0:P], bv_sb[:],
                         start=True, stop=True)
        bvb_sb = const.tile([P, 256], BF16)
        nc.vector.tensor_copy(bvb_sb[:], bb_ps[:])
        qt_sb = big.tile([P, 2, 2048], BF16)    # [hd-of-pair, hp, tokens]
        kt_sb = big.tile([P, 2, 2048], BF16)
        va_sb = big.tile([P, 16, 260], BF16)    # [key, kb, 4x(64 v + 1 one)]
        ctxT_sb = big.tile([P, 2, 2048], BF16)  # [hd-of-pair, kt, tokens]
        nc.gpsimd.memset(
            va_sb.rearrange("p k (s c) -> p k s c", c=65)[:, :, :, 64:65], 1.0)

        # ---------- emission machinery ----------
        import collections
        fillers = collections.deque()
        lateq = collections.deque()     # deferrable work (output projection)
        pending = collections.deque()   # (qb, pair, et, first, last)

        def drain(n):
            for _ in range(n):
                if fillers:
                    fillers.popleft()()

        def drain_late(n):
            for _ in range(n):
                if lateq:
                    lateq.popleft()()

        # ---------- phase pieces ----------
        # projection chains run in the aux PSUM pool so that, when dispersed
        # between attention steps, they never starve the score-tile slots
        def proj_qk(w_sb, b_sb, dst_sb, nch, hp, pool, tag):
            pt = pool.tile([P, 512], F32, tag=tag, name=f"pp{nch}_{hp}")
            for kt in range(8):
                nc.tensor.matmul(
                    pt[:], w_sb[:, kt, 128 * hp:128 * hp + 128],
                    xt_sb[:, kt, 512 * nch:512 * nch + 512],
                    start=(kt == 0), stop=(kt == 7))
            nc.vector.tensor_scalar_add(
                dst_sb[:, hp, 512 * nch:512 * nch + 512], pt[:],
                b_sb[:, hp:hp + 1])

        def proj_v(tt, pool, tag):
            pt = pool.tile([P, 256], F32, tag=tag, name=f"pv{tt}")
            for kt in range(8):
                nc.tensor.matmul(
                    pt[:], xt_sb[:, kt, 128 * tt:128 * tt + 128],
                    wv_sb[:, kt, :], start=(kt == 0), stop=(kt == 7))
            nc.vector.tensor_tensor(
                va_sb.rearrange("p k (s c) -> p k s c", c=65)[:, tt, :, 0:64],
                pt.rearrange("p (s c) -> p s c", c=64),
                bvb_sb.rearrange("p (s c) -> p s c", c=64),
                mybir.AluOpType.add)

        def group_chains(nch, pool, tag):
            """All projection chains needed by q-block group `nch`."""
            ch = [lambda hp=hp: proj_qk(wk_sb, bk_sb, kt_sb, nch, hp,
                                        pool, tag) for hp in range(2)]
            ch += [lambda tt=tt: proj_v(tt, pool, tag)
                   for tt in range(4 * nch, 4 * nch + 4)]
            ch += [lambda hp=hp: proj_qk(wq_sb, bq_sb, qt_sb, nch, hp,
                                         pool, tag) for hp in range(2)]
            return ch

        projq = collections.deque()   # (group, chain-closure)

        def drain_proj(n):
            for _ in range(n):
                if projq:
                    projq.popleft()[1]()

        def force_proj(g):
            while projq and projq[0][0] <= g:
                projq.popleft()[1]()

        ctx_tiles = {}

        def emit_step(qb, pair):
            """Scores + exp (+ diag mask) for a kb-pair of one 128-q block."""
            sc = ps_sc.tile([P, 1024], F32, tag="sc",
                            name=f"sc{qb}_{pair[0]}")
            for u, kb in enumerate(pair):
                for s in range(4):
                    hb, hp = 64 * (s % 2), s // 2
                    nc.tensor.matmul(
                        sc[:, _col(u, s):_col(u, s) + 128],
                        kt_sb[hb:hb + 64, hp, 128 * kb:128 * kb + 128],
                        qt_sb[hb:hb + 64, hp, 128 * qb:128 * qb + 128],
                        start=True, stop=True)
            et = etp.tile([P, 1024], BF16, tag="et", name=f"et{qb}_{pair[0]}")
            if len(pair) == 2:
                nc.scalar.activation(et[:], sc[:], fexp)
            else:
                ap = sc.rearrange("p (b u c) -> p b u c", b=2, u=2)[:, :, 0, :]
                ep = et.rearrange("p (b u c) -> p b u c", b=2, u=2)[:, :, 0, :]
                nc.scalar.activation(ep, ap, fexp)
            if causal and pair[-1] == qb:
                u = len(pair) - 1
                nc.vector.tensor_tensor(
                    et[:, 256 * u:256 * u + 256],
                    et[:, 256 * u:256 * u + 256], md_sb[:], mult)
                nc.vector.tensor_tensor(
                    et[:, 512 + 256 * u:512 + 256 * u + 256],
                    et[:, 512 + 256 * u:512 + 256 * u + 256], md_sb[:], mult)
            pending.append((qb, pair, et))
            if len(pending) > LAG:
                emit_ctx(*pending.popleft())
            drain(K_FILL)
            drain_proj(1)
            if qb >= 10 and not projq:
                drain_late(3)

        def emit_ctx(qb, pair, et):
            nkb = qb + 1 if causal else 16
            if qb not in ctx_tiles:
                ctx_tiles[qb] = ps_ctx.tile([P, 260], F32, tag="ctx",
                                            name=f"ctx{qb}")
            cx = ctx_tiles[qb]
            for u, kb in enumerate(pair):
                for s in range(4):
                    nc.tensor.matmul(
                        cx[:, 65 * s:65 * s + 65],
                        et[:, _col(u, s):_col(u, s) + 128],
                        va_sb[:, kb, 65 * s:65 * s + 65],
                        start=(kb == 0 and s == 0),
                        stop=(kb == nkb - 1 and s == 3))
            if pair[-1] == nkb - 1:
                push_normalize(qb)

        def push_normalize(qb):
            cx = ctx_tiles[qb]
            cq = cqp.tile([P, 256], BF16, tag="cq", name=f"cq{qb}")
            recip = cqp.tile([P, 4], F32, tag="recip", name=f"rc{qb}")

            def f_recip():
                nc.vector.reciprocal(
                    recip[:],
                    cx.rearrange("p (s c) -> p s c", c=65)[:, :, 64])
            fillers.append(f_recip)
            for s in range(4):
                def f_mul(s=s):
                    nc.vector.tensor_scalar_mul(
                        cq[:, 64 * s:64 * s + 64],
                        cx[:, 65 * s:65 * s + 64], recip[:, s:s + 1])
                fillers.append(f_mul)
            for hp in range(2):
                def f_tr(hp=hp):
                    tr = ps_aux.tile([P, P], BF16, tag="aux",
                                     name=f"tr{qb}_{hp}")
                    nc.tensor.transpose(tr[:], cq[:, 128 * hp:128 * hp + 128],
                                        id_sb[:])
                    nc.vector.tensor_copy(
                        ctxT_sb[:, hp, 128 * qb:128 * qb + 128], tr[:])
                    if hp == 1:
                        # ctxT for this q-block (pair) is now fully emitted ->
                        # its output projection may be scheduled (lateq).
                        # The final pair is split per q-block so the first
                        # half drains during the last q-block's steps.
                        if qb % 2 == 1 and qb < 14:
                            push_oproj(256 * (qb // 2), 256)
                        elif qb >= 14:
                            push_oproj(128 * qb, 128)
                fillers.append(f_tr)

        def push_oproj(off, w):
            ob = obp.tile([P, 8, 256], F32, tag="ob", name=f"ob{off}")
            for m in range(8):
                def f_mm(m=m):
                    po = ps_aux.tile([P, 256], F32, tag="aux",
                                     name=f"po{off}_{m}")
                    for kt in range(2):
                        nc.tensor.matmul(
                            po[:, 0:w], wo_sb[:, kt, 128 * m:128 * m + 128],
                            ctxT_sb[:, kt, off:off + w],
                            start=(kt == 0), stop=(kt == 1))
                    nc.vector.tensor_copy(ob[:, m, 0:w], po[:, 0:w])
                lateq.append(f_mm)

            def f_dma():
                nc.sync.dma_start(
                    outT.rearrange("(m p) t -> p m t", p=P)
                    [:, :, off:off + w], ob[:, :, 0:w])
            lateq.append(f_dma)

        # ---------- schedule ----------
        def steps_of(qb):
            nkb = qb + 1 if causal else 16
            kbs = list(range(nkb))
            return [tuple(kbs[i:i + 2]) for i in range(0, nkb, 2)]

        if causal:
            # group 0's projections must run up front; later groups' chains
            # are dispersed between attention steps (aux pool) so the Act
            # engine's exp stream never starves while the PE does proj work
            for ch in group_chains(0, ps_sc, "sc"):
                ch()
                drain(1)
            for nch in range(1, 4):
                for ch in group_chains(nch, ps_aux, "aux"):
                    projq.append((nch, ch))
            for nch in range(4):
                force_proj(nch)
                for qb in range(4 * nch, 4 * nch + 4):
                    for pr in steps_of(qb):
                        emit_step(qb, pr)
        else:
            for nch in range(4):
                for ch in group_chains(nch, ps_sc, "sc"):
                    ch()
                    drain(1)
            for qb in range(16):
                for pr in steps_of(qb):
                    emit_step(qb, pr)

        while pending:
            emit_ctx(*pending.popleft())
        while fillers:
            fillers.popleft()()
        while lateq:
            lateq.popleft()()

    nc.compile()
    return nc


_CACHE = {}


def _get_nc(causal: bool):
    key = bool(causal)
    if key not in _CACHE:
        _CACHE[key] = _emit(key)
    return _CACHE[key]


def _mask():
    """Multiplicative causal mask for a diagonal 128x128 block, replicated
    across the 2 head slots that share a 256-col region."""
    i = np.arange(128)[:, None]
    j = np.arange(128)[None, :]
    m = (j >= i).astype(np.float32)
    return np.tile(m, (1, 2)).astype(NPBF)


def kernel(**inputs):
    x = np.asarray(inputs["x"], dtype=np.float32)
    Wq = np.asarray(inputs["Wq"], dtype=np.float32)
    bq = np.asarray(inputs["bq"], dtype=np.float32)
    Wk = np.asarray(inputs["Wk"], dtype=np.float32)
    bk = np.asarray(inputs["bk"], dtype=np.float32)
    Wv = np.asarray(inputs["Wv"], dtype=np.float32)
    bv = np.asarray(inputs["bv"], dtype=np.float32)
    Wo = np.asarray(inputs["Wo"], dtype=np.float32)
    bo = np.asarray(inputs["bo"], dtype=np.float32)
    causal = bool(int(np.asarray(inputs["enable_causal"])))

    scale = np.float32(1.0 / np.sqrt(HD))
    wqT = Wq.T                    # [in, out]
    wkT = (Wk * scale).T
    wvT = Wv.T
    woT = Wo.T                    # [ctx-dim, out]
    bks = bk * scale

    xs = []
    for b in range(B):
        xt = x[b].T.reshape(8, 128, 4, 512).transpose(2, 1, 0, 3)
        xs.append(np.ascontiguousarray(xt.astype(NPBF)))

    ident = np.eye(P, dtype=NPBF)
    if causal:
        md = _mask()

    nc = _get_nc(causal)
    in_maps = []
    for c in range(NC):
        b, g = divmod(c, 4)
        cols = slice(256 * g, 256 * g + 256)
        wq_r = np.ascontiguousarray(
            wqT[:, cols].reshape(8, 128, 256).transpose(1, 0, 2)).astype(NPBF)
        wk_r = np.ascontiguousarray(
            wkT[:, cols].reshape(8, 128, 256).transpose(1, 0, 2)).astype(NPBF)
        wv_r = np.ascontiguousarray(
            wvT[:, cols].reshape(8, 128, 256).transpose(1, 0, 2)).astype(NPBF)
        wo_r = np.ascontiguousarray(
            woT[cols, :].reshape(2, 128, 1024).transpose(1, 0, 2)).astype(NPBF)
        m = {"xT4": xs[b],
             "wq": wq_r, "wk": wk_r, "wv": wv_r, "wo": wo_r,
             "bq": np.ascontiguousarray(bq[cols].reshape(2, 128).T),
             "bk": np.ascontiguousarray(bks[cols].reshape(2, 128).T),
             "bv": np.ascontiguousarray(bv[cols].reshape(1, 256)).astype(NPBF),
             "ident": ident}
        if causal:
            m["md"] = md
        in_maps.append(m)

    global LAST_RESULT
    res = run_bass_kernel_spmd(nc, in_maps, list(range(NC)), trace=TRACE)
    LAST_RESULT = res

    # unshard: sum the 4 head-group partials per batch (row-parallel Wo), +bo
    out = np.empty((B, S, D), dtype=np.float32)
    for b in range(B):
        acc = res.results[4 * b]["outT"].astype(np.float32)
        for g in range(1, 4):
            acc = acc + res.results[4 * b + g]["outT"]
        out[b] = acc.T + bo[None, :]
    return out


# revision 49
# speedup vs baseline: 5.8024x; 1.0009x over previous
"""Trainium2 Bass kernel for nn_MultiHeadAttention (B=2, S=2048, D=1024, H=16, causal).

Sharding across 8 NeuronCores (single SPMD program, head-parallel TP):
  - Core c owns batch b=c//4 and head group g=c%4 (4 heads = 256 of the 1024
    projection columns).  W_q/W_k/W_v are column-sharded, W_o row-sharded.
  - Each core projects Q/K/V for ALL 2048 tokens of its batch but only its 4
    heads, runs full causal attention for those heads entirely in SBUF (no
    K/V exchange => ZERO collectives), then computes its partial output
    projection out_partial = ctx_heads @ Wo_rows.  The host unshard step sums
    the 4 partial outputs per batch and adds bo (the row-parallel reduction
    of tensor-parallel attention, folded into the host-side gather that the
    full-IO contract already requires).
  - bf16 operands everywhere on the PE (1 cycle/row at any N); f32 PSUM
    accumulation; 1/sqrt(64) folded into Wk/bk on the host.
  - Attention runs as a flat software-pipelined stream of (qb, kb-pair)
    steps at 128-query granularity: transposed scores scoresT[k, q] for all
    4 heads of two key-blocks land in one [128,1024] PSUM tile (parity-
    grouped so each PSUM bank only sees one PE tile row position - HW
    constraint), a single Act exp covers the pair, causal masking is
    multiplicative on the exp'd tile (diagonal blocks only), and the
    context matmuls are q-major (stationary = exp tile, moving = V plus a
    ones-column that yields the softmax denominator as column 64).  The
    denominator is then a per-partition scalar, so normalization is plain
    tensor_scalar multiplies; a PE transpose packs the normalized context
    back to hd-major for the output projection.
  - ctx matmuls lag the score stream (software pipelining) and the
    normalize/transpose/output-projection work is spread as filler between
    later steps, so PE, Act and DVE stay concurrently busy.
"""
import numpy as np
import ml_dtypes

import concourse.bass as bass
import concourse.bacc as bacc
import concourse.mybir as mybir
import concourse.tile as tile
from concourse.bass_utils import run_bass_kernel_spmd

B, S, D, H, HD = 2, 2048, 1024, 16, 64
NC = 8
P = 128
F32 = mybir.dt.float32
BF16 = mybir.dt.bfloat16
NPBF = ml_dtypes.bfloat16

TRACE = False        # set True (e.g. from test.py) to capture an NTFF profile
LAST_RESULT = None   # BassKernelResults of the most recent kernel() call

LAG = 5              # ctx stream lags the score stream by this many pair-steps
K_FILL = 3           # filler items drained per pair-step


def _col(u, s):
    """Column of head-slot s, pair-position u in the [128,1024] score tile.

    Parity-grouped: bank 0 (cols 0:512) holds even heads (PE row base 0),
    bank 1 (cols 512:1024) odd heads (row base 64) - matmuls into one PSUM
    bank must share a single PE tile row position.
    """
    return (0 if s % 2 == 0 else 512) + 256 * u + (128 if s >= 2 else 0)


def _emit(causal: bool):
    nc = bacc.Bacc(trn_type="TRN2", num_devices=NC)
    fexp = mybir.ActivationFunctionType.Exp
    mult = mybir.AluOpType.mult

    # ---- per-core DRAM inputs (host pre-sharded / pre-transposed) ----
    xT4 = nc.dram_tensor("xT4", [4, P, 8, 512], BF16, kind="ExternalInput")
    wq_d = nc.dram_tensor("wq", [P, 8, 256], BF16, kind="ExternalInput")
    wk_d = nc.dram_tensor("wk", [P, 8, 256], BF16, kind="ExternalInput")
    wv_d = nc.dram_tensor("wv", [P, 8, 256], BF16, kind="ExternalInput")
    wo_d = nc.dram_tensor("wo", [P, 2, 1024], BF16, kind="ExternalInput")
    bq_d = nc.dram_tensor("bq", [P, 2], F32, kind="ExternalInput")
    bk_d = nc.dram_tensor("bk", [P, 2], F32, kind="ExternalInput")
    bv_d = nc.dram_tensor("bv", [1, 256], BF16, kind="ExternalInput")
    id_d = nc.dram_tensor("ident", [P, P], BF16, kind="ExternalInput")
    if causal:
        md_d = nc.dram_tensor("md", [P, 256], BF16, kind="ExternalInput")
    outT = nc.dram_tensor("outT", [D, S], F32, kind="ExternalOutput")

    with tile.TileContext(nc) as tc, \
         tc.tile_pool(name="const", bufs=1) as const, \
         tc.tile_pool(name="w", bufs=1) as wpool, \
         tc.tile_pool(name="big", bufs=1) as big, \
         tc.tile_pool(name="et", bufs=6) as etp, \
         tc.tile_pool(name="cq", bufs=2) as cqp, \
         tc.tile_pool(name="ob", bufs=2) as obp, \
         tc.tile_pool(name="ps_sc", bufs=2, space="PSUM") as ps_sc, \
         tc.tile_pool(name="ps_ctx", bufs=2, space="PSUM") as ps_ctx, \
         tc.tile_pool(name="ps_aux", bufs=2, space="PSUM") as ps_aux:

        # ---------- constants / weights (critical-path DMA order) ----------
        wk_sb = wpool.tile([P, 8, 256], BF16)
        nc.sync.dma_start(wk_sb[:], wk_d[:])
        bk_sb = const.tile([P, 2], F32)
        nc.sync.dma_start(bk_sb[:], bk_d[:])
        xt_sb = big.tile([P, 8, 2048], BF16)
        for o in range(8):  # per-o so the first proj chain starts ASAP
            nc.sync.dma_start(xt_sb[:, o, 0:512], xT4[0, :, o, :])
        wv_sb = wpool.tile([P, 8, 256], BF16)
        nc.sync.dma_start(wv_sb[:], wv_d[:])
        bv_sb = const.tile([1, 256], BF16)
        nc.sync.dma_start(bv_sb[:], bv_d[:])
        wq_sb = wpool.tile([P, 8, 256], BF16)
        nc.sync.dma_start(wq_sb[:], wq_d[:])
        bq_sb = const.tile([P, 2], F32)
        nc.sync.dma_start(bq_sb[:], bq_d[:])
        if causal:
            md_sb = const.tile([P, 256], BF16)
            nc.sync.dma_start(md_sb[:], md_d[:])
        id_sb = const.tile([P, P], BF16)
        nc.sync.dma_start(id_sb[:], id_d[:])
        wo_sb = wpool.tile([P, 2, 1024], BF16)
        nc.sync.dma_start(wo_sb[:], wo_d[:])
        for nch in range(1, 4):
            nc.sync.dma_start(xt_sb[:, :, 512 * nch:512 * nch + 512],
                              xT4[nch, :, :, :])

        ones_sb = const.tile([P, P], BF16)
        nc.gpsimd.memset(ones_sb[:], 1.0)
        # bv broadcast to all partitions once; folded into the V copy as a
        # DVE add instead of 16 per-tile bias matmuls
        bb_ps = ps_aux.tile([P, 256], F32, tag="aux", name="bb")
        nc.tensor.matmul(bb_ps[:], ones_sb[0:1, 0:P], bv_sb[:],
                         start=True, stop=True)
        bvb_sb = const.tile([P, 256], BF16)
        nc.vector.tensor_copy(bvb_sb[:], bb_ps[:])
        qt_sb = big.tile([P, 2, 2048], BF16)    # [hd-of-pair, hp, tokens]
        kt_sb = big.tile([P, 2, 2048], BF16)
        va_sb = big.tile([P, 16, 260], BF16)    # [key, kb, 4x(64 v + 1 one)]
        ctxT_sb = big.tile([P, 2, 2048], BF16)  # [hd-of-pair, kt, tokens]
        nc.gpsimd.memset(
            va_sb.rearrange("p k (s c) -> p k s c", c=65)[:, :, :, 64:65], 1.0)

        # ---------- emission machinery ----------
        import collections
        fillers = collections.deque()
        lateq = collections.deque()     # deferrable work (output projection)
        pending = collections.deque()   # (qb, pair, et, first, last)

        def drain(n):
            for _ in range(n):
                if fillers:
                    fillers.popleft()()

        def drain_late(n):
            for _ in range(n):
                if lateq:
                    lateq.popleft()()

        # ---------- phase pieces ----------
        # projection chains run in the aux PSUM pool so that, when dispersed
        # between attention steps, they never starve the score-tile slots
        def proj_qk(w_sb, b_sb, dst_sb, nch, hp, pool, tag):
            pt = pool.tile([P, 512], F32, tag=tag, name=f"pp{nch}_{hp}")
            for kt in range(8):
                nc.tensor.matmul(
                    pt[:], w_sb[:, kt, 128 * hp:128 * hp + 128],
                    xt_sb[:, kt, 512 * nch:512 * nch + 512],
                    start=(kt == 0), stop=(kt == 7))
            nc.vector.tensor_scalar_add(
                dst_sb[:, hp, 512 * nch:512 * nch + 512], pt[:],
                b_sb[:, hp:hp + 1])

        def proj_v(tt, pool, tag):
            pt = pool.tile([P, 256], F32, tag=tag, name=f"pv{tt}")
            for kt in range(8):
                nc.tensor.matmul(
                    pt[:], xt_sb[:, kt, 128 * tt:128 * tt + 128],
                    wv_sb[:, kt, :], start=(kt == 0), stop=(kt == 7))
            nc.vector.tensor_tensor(
                va_sb.rearrange("p k (s c) -> p k s c", c=65)[:, tt, :, 0:64],
                pt.rearrange("p (s c) -> p s c", c=64),
                bvb_sb.rearrange("p (s c) -> p s c", c=64),
                mybir.AluOpType.add)

        def group_chains(nch, pool, tag):
            """All projection chains needed by q-block group `nch`."""
            ch = [lambda hp=hp: proj_qk(wk_sb, bk_sb, kt_sb, nch, hp,
                                        pool, tag) for hp in range(2)]
            ch += [lambda tt=tt: proj_v(tt, pool, tag)
                   for tt in range(4 * nch, 4 * nch + 4)]
            ch += [lambda hp=hp: proj_qk(wq_sb, bq_sb, qt_sb, nch, hp,
                                         pool, tag) for hp in range(2)]
            return ch

        projq = collections.deque()   # (group, chain-closure)

        def drain_proj(n):
            for _ in range(n):
                if projq:
                    projq.popleft()[1]()

        def force_proj(g):
            while projq and projq[0][0] <= g:
                projq.popleft()[1]()

        ctx_tiles = {}

        def emit_step(qb, pair):
            """Scores + exp (+ diag mask) for a kb-pair of one 128-q block."""
            sc = ps_sc.tile([P, 1024], F32, tag="sc",
                            name=f"sc{qb}_{pair[0]}")
            for u, kb in enumerate(pair):
                for s in range(4):
                    hb, hp = 64 * (s % 2), s // 2
                    nc.tensor.matmul(
                        sc[:, _col(u, s):_col(u, s) + 128],
                        kt_sb[hb:hb + 64, hp, 128 * kb:128 * kb + 128],
                        qt_sb[hb:hb + 64, hp, 128 * qb:128 * qb + 128],
                        start=True, stop=True)
            et = etp.tile([P, 1024], BF16, tag="et", name=f"et{qb}_{pair[0]}")
            if len(pair) == 2:
                nc.scalar.activation(et[:], sc[:], fexp)
            else:
                ap = sc.rearrange("p (b u c) -> p b u c", b=2, u=2)[:, :, 0, :]
                ep = et.rearrange("p (b u c) -> p b u c", b=2, u=2)[:, :, 0, :]
                nc.scalar.activation(ep, ap, fexp)
            if causal and pair[-1] == qb:
                u = len(pair) - 1
                nc.vector.tensor_tensor(
                    et[:, 256 * u:256 * u + 256],
                    et[:, 256 * u:256 * u + 256], md_sb[:], mult)
                nc.vector.tensor_tensor(
                    et[:, 512 + 256 * u:512 + 256 * u + 256],
                    et[:, 512 + 256 * u:512 + 256 * u + 256], md_sb[:], mult)
            pending.append((qb, pair, et))
            if len(pending) > LAG:
                emit_ctx(*pending.popleft())
            drain(K_FILL)
            drain_proj(1)
            if qb >= 10 and not projq:
                drain_late(3)

        def emit_ctx(qb, pair, et):
            nkb = qb + 1 if causal else 16
            if qb not in ctx_tiles:
                ctx_tiles[qb] = ps_ctx.tile([P, 260], F32, tag="ctx",
                                            name=f"ctx{qb}")
            cx = ctx_tiles[qb]
            for u, kb in enumerate(pair):
                for s in range(4):
                    nc.tensor.matmul(
                        cx[:, 65 * s:65 * s + 65],
                        et[:, _col(u, s):_col(u, s) + 128],
                        va_sb[:, kb, 65 * s:65 * s + 65],
                        start=(kb == 0 and s == 0),
                        stop=(kb == nkb - 1 and s == 3))
            if pair[-1] == nkb - 1:
                push_normalize(qb)

        def push_normalize(qb):
            cx = ctx_tiles[qb]
            cq = cqp.tile([P, 256], BF16, tag="cq", name=f"cq{qb}")
            recip = cqp.tile([P, 4], F32, tag="recip", name=f"rc{qb}")

            def f_recip():
                nc.vector.reciprocal(
                    recip[:],
                    cx.rearrange("p (s c) -> p s c", c=65)[:, :, 64])
            fillers.append(f_recip)
            for s in range(4):
                def f_mul(s=s):
                    nc.vector.tensor_scalar_mul(
                        cq[:, 64 * s:64 * s + 64],
                        cx[:, 65 * s:65 * s + 64], recip[:, s:s + 1])
                fillers.append(f_mul)
            for hp in range(2):
                def f_tr(hp=hp):
                    tr = ps_aux.tile([P, P], BF16, tag="aux",
                                     name=f"tr{qb}_{hp}")
                    nc.tensor.transpose(tr[:], cq[:, 128 * hp:128 * hp + 128],
                                        id_sb[:])
                    nc.vector.tensor_copy(
                        ctxT_sb[:, hp, 128 * qb:128 * qb + 128], tr[:])
                    if hp == 1:
                        # ctxT for this q-block (pair) is now fully emitted ->
                        # its output projection may be scheduled (lateq).
                        # The final pair is split per q-block so the first
                        # half drains during the last q-block's steps.
                        if qb % 2 == 1 and qb < 14:
                            push_oproj(256 * (qb // 2), 256)
                        elif qb >= 14:
                            push_oproj(128 * qb, 128)
                fillers.append(f_tr)

        def push_oproj(off, w):
            ob = obp.tile([P, 8, 256], F32, tag="ob", name=f"ob{off}")
            for m in range(8):
                def f_mm(m=m):
                    po = ps_aux.tile([P, 256], F32, tag="aux",
                                     name=f"po{off}_{m}")
                    for kt in range(2):
                        nc.tensor.matmul(
                            po[:, 0:w], wo_sb[:, kt, 128 * m:128 * m + 128],
                            ctxT_sb[:, kt, off:off + w],
                            start=(kt == 0), stop=(kt == 1))
                    nc.vector.tensor_copy(ob[:, m, 0:w], po[:, 0:w])
                lateq.append(f_mm)

            def f_dma():
                nc.sync.dma_start(
                    outT.rearrange("(m p) t -> p m t", p=P)
                    [:, :, off:off + w], ob[:, :, 0:w])
            lateq.append(f_dma)

        # ---------- schedule ----------
        def steps_of(qb):
            nkb = qb + 1 if causal else 16
            kbs = list(range(nkb))
            return [tuple(kbs[i:i + 2]) for i in range(0, nkb, 2)]

        if causal:
            # group 0's projections must run up front; later groups' chains
            # are dispersed between attention steps (aux pool) so the Act
            # engine's exp stream never starves while the PE does proj work
            for ch in group_chains(0, ps_sc, "sc"):
                ch()
                drain(1)
            for nch in range(1, 4):
                for ch in group_chains(nch, ps_aux, "aux"):
                    projq.append((nch, ch))
            for nch in range(4):
                force_proj(nch)
                for qb in range(4 * nch, 4 * nch + 4):
                    for pr in steps_of(qb):
                        emit_step(qb, pr)
        else:
            for nch in range(4):
                for ch in group_chains(nch, ps_sc, "sc"):
                    ch()
                    drain(1)
            for qb in range(16):
                for pr in steps_of(qb):
                    emit_step(qb, pr)

        while pending:
            emit_ctx(*pending.popleft())
        while fillers:
            fillers.popleft()()
        while lateq:
            lateq.popleft()()

    nc.compile()
    return nc


_CACHE = {}


def _get_nc(causal: bool):
    key = bool(causal)
    if key not in _CACHE:
        _CACHE[key] = _emit(key)
    return _CACHE[key]


def _mask():
    """Multiplicative causal mask for a diagonal 128x128 block, replicated
    across the 2 head slots that share a 256-col region."""
    i = np.arange(128)[:, None]
    j = np.arange(128)[None, :]
    m = (j >= i).astype(np.float32)
    return np.tile(m, (1, 2)).astype(NPBF)


def kernel(**inputs):
    x = np.asarray(inputs["x"], dtype=np.float32)
    Wq = np.asarray(inputs["Wq"], dtype=np.float32)
    bq = np.asarray(inputs["bq"], dtype=np.float32)
    Wk = np.asarray(inputs["Wk"], dtype=np.float32)
    bk = np.asarray(inputs["bk"], dtype=np.float32)
    Wv = np.asarray(inputs["Wv"], dtype=np.float32)
    bv = np.asarray(inputs["bv"], dtype=np.float32)
    Wo = np.asarray(inputs["Wo"], dtype=np.float32)
    bo = np.asarray(inputs["bo"], dtype=np.float32)
    causal = bool(int(np.asarray(inputs["enable_causal"])))

    scale = np.float32(1.0 / np.sqrt(HD))
    wqT = Wq.T                    # [in, out]
    wkT = (Wk * scale).T
    wvT = Wv.T
    woT = Wo.T                    # [ctx-dim, out]
    bks = bk * scale

    xs = []
    for b in range(B):
        xt = x[b].T.reshape(8, 128, 4, 512).transpose(2, 1, 0, 3)
        xs.append(np.ascontiguousarray(xt.astype(NPBF)))

    ident = np.eye(P, dtype=NPBF)
    if causal:
        md = _mask()

    nc = _get_nc(causal)
    in_maps = []
    for c in range(NC):
        b, g = divmod(c, 4)
        cols = slice(256 * g, 256 * g + 256)
        wq_r = np.ascontiguousarray(
            wqT[:, cols].reshape(8, 128, 256).transpose(1, 0, 2)).astype(NPBF)
        wk_r = np.ascontiguousarray(
            wkT[:, cols].reshape(8, 128, 256).transpose(1, 0, 2)).astype(NPBF)
        wv_r = np.ascontiguousarray(
            wvT[:, cols].reshape(8, 128, 256).transpose(1, 0, 2)).astype(NPBF)
        wo_r = np.ascontiguousarray(
            woT[cols, :].reshape(2, 128, 1024).transpose(1, 0, 2)).astype(NPBF)
        m = {"xT4": xs[b],
             "wq": wq_r, "wk": wk_r, "wv": wv_r, "wo": wo_r,
             "bq": np.ascontiguousarray(bq[cols].reshape(2, 128).T),
             "bk": np.ascontiguousarray(bks[cols].reshape(2, 128).T),
             "bv": np.ascontiguousarray(bv[cols].reshape(1, 256)).astype(NPBF),
             "ident": ident}
        if causal:
            m["md"] = md
        in_maps.append(m)

    global LAST_RESULT
    res = run_bass_kernel_spmd(nc, in_maps, list(range(NC)), trace=TRACE)
    LAST_RESULT = res

    # unshard: sum the 4 head-group partials per batch (row-parallel Wo), +bo
    out = np.empty((B, S, D), dtype=np.float32)
    for b in range(B):
        acc = res.results[4 * b]["outT"].astype(np.float32)
        for g in range(1, 4):
            acc = acc + res.results[4 * b + g]["outT"]
        out[b] = acc.T + bo[None, :]
    return out


# revision 53
# speedup vs baseline: 5.8894x; 1.0150x over previous
"""Trainium2 Bass kernel for nn_MultiHeadAttention (B=2, S=2048, D=1024, H=16, causal).

Sharding across 8 NeuronCores (single SPMD program, head-parallel TP):
  - Core c owns batch b=c//4 and head group g=c%4 (4 heads = 256 of the 1024
    projection columns).  W_q/W_k/W_v are column-sharded, W_o row-sharded.
  - Each core projects Q/K/V for ALL 2048 tokens of its batch but only its 4
    heads, runs full causal attention for those heads entirely in SBUF (no
    K/V exchange => ZERO collectives), then computes its partial output
    projection out_partial = ctx_heads @ Wo_rows.  The host unshard step sums
    the 4 partial outputs per batch and adds bo (the row-parallel reduction
    of tensor-parallel attention, folded into the host-side gather that the
    full-IO contract already requires).
  - bf16 operands everywhere on the PE (1 cycle/row at any N); f32 PSUM
    accumulation; 1/sqrt(64) folded into Wk/bk on the host.
  - Attention runs as a flat software-pipelined stream of (qb, kb-pair)
    steps at 128-query granularity: transposed scores scoresT[k, q] for all
    4 heads of two key-blocks land in one [128,1024] PSUM tile (parity-
    grouped so each PSUM bank only sees one PE tile row position - HW
    constraint), a single Act exp covers the pair, causal masking is
    multiplicative on the exp'd tile (diagonal blocks only), and the
    context matmuls are q-major (stationary = exp tile, moving = V plus a
    ones-column that yields the softmax denominator as column 64).  The
    denominator is then a per-partition scalar, so normalization is plain
    tensor_scalar multiplies; a PE transpose packs the normalized context
    back to hd-major for the output projection.
  - ctx matmuls lag the score stream (software pipelining) and the
    normalize/transpose/output-projection work is spread as filler between
    later steps, so PE, Act and DVE stay concurrently busy.
"""
import numpy as np
import ml_dtypes

import concourse.bass as bass
import concourse.bacc as bacc
import concourse.mybir as mybir
import concourse.tile as tile
from concourse.bass_utils import run_bass_kernel_spmd

B, S, D, H, HD = 2, 2048, 1024, 16, 64
NC = 8
P = 128
F32 = mybir.dt.float32
BF16 = mybir.dt.bfloat16
NPBF = ml_dtypes.bfloat16

TRACE = False        # set True (e.g. from test.py) to capture an NTFF profile
LAST_RESULT = None   # BassKernelResults of the most recent kernel() call

LAG = 5              # ctx stream lags the score stream by this many pair-steps
K_FILL = 3           # filler items drained per pair-step


def _col(u, s):
    """Column of head-slot s, pair-position u in the [128,1024] score tile.

    Parity-grouped: bank 0 (cols 0:512) holds even heads (PE row base 0),
    bank 1 (cols 512:1024) odd heads (row base 64) - matmuls into one PSUM
    bank must share a single PE tile row position.
    """
    return (0 if s % 2 == 0 else 512) + 256 * u + (128 if s >= 2 else 0)


def _emit(causal: bool):
    nc = bacc.Bacc(trn_type="TRN2", num_devices=NC)
    fexp = mybir.ActivationFunctionType.Exp
    mult = mybir.AluOpType.mult

    # ---- per-core DRAM inputs (host pre-sharded / pre-transposed) ----
    xT4 = nc.dram_tensor("xT4", [4, P, 8, 512], BF16, kind="ExternalInput")
    wq_d = nc.dram_tensor("wq", [P, 8, 256], BF16, kind="ExternalInput")
    wk_d = nc.dram_tensor("wk", [P, 8, 256], BF16, kind="ExternalInput")
    wv_d = nc.dram_tensor("wv", [P, 8, 256], BF16, kind="ExternalInput")
    wo_d = nc.dram_tensor("wo", [P, 2, 1024], BF16, kind="ExternalInput")
    bq_d = nc.dram_tensor("bq", [P, 2], F32, kind="ExternalInput")
    bk_d = nc.dram_tensor("bk", [P, 2], F32, kind="ExternalInput")
    bv_d = nc.dram_tensor("bv", [1, 256], BF16, kind="ExternalInput")
    id_d = nc.dram_tensor("ident", [P, P], BF16, kind="ExternalInput")
    if causal:
        md_d = nc.dram_tensor("md", [P, 256], BF16, kind="ExternalInput")
    outT = nc.dram_tensor("outT", [D, S], F32, kind="ExternalOutput")

    with tile.TileContext(nc) as tc, \
         tc.tile_pool(name="const", bufs=1) as const, \
         tc.tile_pool(name="w", bufs=1) as wpool, \
         tc.tile_pool(name="big", bufs=1) as big, \
         tc.tile_pool(name="et", bufs=6) as etp, \
         tc.tile_pool(name="cq", bufs=2) as cqp, \
         tc.tile_pool(name="ob", bufs=2) as obp, \
         tc.tile_pool(name="ps_sc", bufs=2, space="PSUM") as ps_sc, \
         tc.tile_pool(name="ps_ctx", bufs=2, space="PSUM") as ps_ctx, \
         tc.tile_pool(name="ps_aux", bufs=2, space="PSUM") as ps_aux:

        # ---------- constants / weights (critical-path DMA order) ----------
        wk_sb = wpool.tile([P, 8, 256], BF16)
        nc.sync.dma_start(wk_sb[:], wk_d[:])
        bk_sb = const.tile([P, 2], F32)
        nc.sync.dma_start(bk_sb[:], bk_d[:])
        xt_sb = big.tile([P, 8, 2048], BF16)
        for o in range(8):  # per-o so the first proj chain starts ASAP
            nc.sync.dma_start(xt_sb[:, o, 0:512], xT4[0, :, o, :])
        wv_sb = wpool.tile([P, 8, 256], BF16)
        nc.sync.dma_start(wv_sb[:], wv_d[:])
        bv_sb = const.tile([1, 256], BF16)
        nc.sync.dma_start(bv_sb[:], bv_d[:])
        wq_sb = wpool.tile([P, 8, 256], BF16)
        nc.sync.dma_start(wq_sb[:], wq_d[:])
        bq_sb = const.tile([P, 2], F32)
        nc.sync.dma_start(bq_sb[:], bq_d[:])
        if causal:
            md_sb = const.tile([P, 256], BF16)
            nc.sync.dma_start(md_sb[:], md_d[:])
        id_sb = const.tile([P, P], BF16)
        nc.sync.dma_start(id_sb[:], id_d[:])
        wo_sb = wpool.tile([P, 2, 1024], BF16)
        nc.sync.dma_start(wo_sb[:], wo_d[:])
        for nch in range(1, 4):
            nc.sync.dma_start(xt_sb[:, :, 512 * nch:512 * nch + 512],
                              xT4[nch, :, :, :])

        ones_sb = const.tile([P, P], BF16)
        nc.gpsimd.memset(ones_sb[:], 1.0)
        # bv broadcast to all partitions once; folded into the V copy as a
        # DVE add instead of 16 per-tile bias matmuls
        bb_ps = ps_aux.tile([P, 256], F32, tag="aux", name="bb")
        nc.tensor.matmul(bb_ps[:], ones_sb[0:1, 0:P], bv_sb[:],
                         start=True, stop=True)
        bvb_sb = const.tile([P, 256], BF16)
        nc.vector.tensor_copy(bvb_sb[:], bb_ps[:])
        qt_sb = big.tile([P, 2, 2048], BF16)    # [hd-of-pair, hp, tokens]
        kt_sb = big.tile([P, 2, 2048], BF16)
        va_sb = big.tile([P, 16, 260], BF16)    # [key, kb, 4x(64 v + 1 one)]
        ctxT_sb = big.tile([P, 2, 2048], BF16)  # [hd-of-pair, kt, tokens]
        nc.gpsimd.memset(
            va_sb.rearrange("p k (s c) -> p k s c", c=65)[:, :, :, 64:65], 1.0)

        # ---------- emission machinery ----------
        import collections
        fillers = collections.deque()
        lateq = collections.deque()     # deferrable work (output projection)
        pending = collections.deque()   # (qb, pair, et, first, last)

        def drain(n):
            for _ in range(n):
                if fillers:
                    fillers.popleft()()

        def drain_late(n):
            for _ in range(n):
                if lateq:
                    lateq.popleft()()

        # ---------- phase pieces ----------
        # projection chains run in the aux PSUM pool so that, when dispersed
        # between attention steps, they never starve the score-tile slots
        def proj_qk(w_sb, b_sb, dst_sb, nch, hp, pool, tag):
            pt = pool.tile([P, 512], F32, tag=tag, name=f"pp{nch}_{hp}")
            for kt in range(8):
                nc.tensor.matmul(
                    pt[:], w_sb[:, kt, 128 * hp:128 * hp + 128],
                    xt_sb[:, kt, 512 * nch:512 * nch + 512],
                    start=(kt == 0), stop=(kt == 7))
            nc.vector.tensor_scalar_add(
                dst_sb[:, hp, 512 * nch:512 * nch + 512], pt[:],
                b_sb[:, hp:hp + 1])

        def proj_v(tt, pool, tag):
            pt = pool.tile([P, 256], F32, tag=tag, name=f"pv{tt}")
            for kt in range(8):
                nc.tensor.matmul(
                    pt[:], xt_sb[:, kt, 128 * tt:128 * tt + 128],
                    wv_sb[:, kt, :], start=(kt == 0), stop=(kt == 7))
            nc.vector.tensor_tensor(
                va_sb.rearrange("p k (s c) -> p k s c", c=65)[:, tt, :, 0:64],
                pt.rearrange("p (s c) -> p s c", c=64),
                bvb_sb.rearrange("p (s c) -> p s c", c=64),
                mybir.AluOpType.add)

        def group_chains(nch, pool, tag):
            """All projection chains needed by q-block group `nch`."""
            ch = [lambda hp=hp: proj_qk(wk_sb, bk_sb, kt_sb, nch, hp,
                                        pool, tag) for hp in range(2)]
            ch += [lambda tt=tt: proj_v(tt, pool, tag)
                   for tt in range(4 * nch, 4 * nch + 4)]
            ch += [lambda hp=hp: proj_qk(wq_sb, bq_sb, qt_sb, nch, hp,
                                         pool, tag) for hp in range(2)]
            return ch

        projq = collections.deque()   # (group, chain-closure)

        def drain_proj(n):
            for _ in range(n):
                if projq:
                    projq.popleft()[1]()

        def force_proj(g):
            while projq and projq[0][0] <= g:
                projq.popleft()[1]()

        ctx_tiles = {}

        def emit_step(items):
            """Scores + exp (+ diag masks) for 1-2 (qb, kb) items; items may
            span a q-block boundary so every exp covers a full pair-tile."""
            qb0, kb0 = items[0]
            sc = ps_sc.tile([P, 1024], F32, tag="sc", name=f"sc{qb0}_{kb0}")
            for u, (qb, kb) in enumerate(items):
                for s in range(4):
                    hb, hp = 64 * (s % 2), s // 2
                    nc.tensor.matmul(
                        sc[:, _col(u, s):_col(u, s) + 128],
                        kt_sb[hb:hb + 64, hp, 128 * kb:128 * kb + 128],
                        qt_sb[hb:hb + 64, hp, 128 * qb:128 * qb + 128],
                        start=True, stop=True)
            et = etp.tile([P, 1024], BF16, tag="et", name=f"et{qb0}_{kb0}")
            if len(items) == 2:
                nc.scalar.activation(et[:], sc[:], fexp)
            else:
                ap = sc.rearrange("p (b u c) -> p b u c", b=2, u=2)[:, :, 0, :]
                ep = et.rearrange("p (b u c) -> p b u c", b=2, u=2)[:, :, 0, :]
                nc.scalar.activation(ep, ap, fexp)
            if causal:
                for u, (qb, kb) in enumerate(items):
                    if kb == qb:   # diagonal block of this q-block
                        nc.vector.tensor_tensor(
                            et[:, 256 * u:256 * u + 256],
                            et[:, 256 * u:256 * u + 256], md_sb[:], mult)
                        nc.vector.tensor_tensor(
                            et[:, 512 + 256 * u:512 + 256 * u + 256],
                            et[:, 512 + 256 * u:512 + 256 * u + 256],
                            md_sb[:], mult)
            pending.append((items, et))
            if len(pending) > LAG:
                emit_ctx(*pending.popleft())
            drain(K_FILL)
            drain_proj(1)
            if items[-1][0] >= 10 and not projq:
                drain_late(3)

        def emit_ctx(items, et):
            for u, (qb, kb) in enumerate(items):
                nkb = qb + 1 if causal else 16
                if qb not in ctx_tiles:
                    ctx_tiles[qb] = ps_ctx.tile([P, 260], F32, tag="ctx",
                                                name=f"ctx{qb}")
                cx = ctx_tiles[qb]
                for s in range(4):
                    nc.tensor.matmul(
                        cx[:, 65 * s:65 * s + 65],
                        et[:, _col(u, s):_col(u, s) + 128],
                        va_sb[:, kb, 65 * s:65 * s + 65],
                        start=(kb == 0 and s == 0),
                        stop=(kb == nkb - 1 and s == 3))
                if kb == nkb - 1:
                    push_normalize(qb)

        def push_normalize(qb):
            cx = ctx_tiles[qb]
            cq = cqp.tile([P, 256], BF16, tag="cq", name=f"cq{qb}")
            recip = cqp.tile([P, 4], F32, tag="recip", name=f"rc{qb}")

            def f_recip():
                nc.vector.reciprocal(
                    recip[:],
                    cx.rearrange("p (s c) -> p s c", c=65)[:, :, 64])
            fillers.append(f_recip)
            for s in range(4):
                def f_mul(s=s):
                    nc.vector.tensor_scalar_mul(
                        cq[:, 64 * s:64 * s + 64],
                        cx[:, 65 * s:65 * s + 64], recip[:, s:s + 1])
                fillers.append(f_mul)
            for hp in range(2):
                def f_tr(hp=hp):
                    tr = ps_aux.tile([P, P], BF16, tag="aux",
                                     name=f"tr{qb}_{hp}")
                    nc.tensor.transpose(tr[:], cq[:, 128 * hp:128 * hp + 128],
                                        id_sb[:])
                    nc.vector.tensor_copy(
                        ctxT_sb[:, hp, 128 * qb:128 * qb + 128], tr[:])
                    if hp == 1:
                        # ctxT for this q-block (pair) is now fully emitted ->
                        # its output projection may be scheduled (lateq).
                        # The final pair is split per q-block so the first
                        # half drains during the last q-block's steps.
                        if qb % 2 == 1 and qb < 14:
                            push_oproj(256 * (qb // 2), 256)
                        elif qb >= 14:
                            push_oproj(128 * qb, 128)
                fillers.append(f_tr)

        def push_oproj(off, w):
            ob = obp.tile([P, 8, 256], F32, tag="ob", name=f"ob{off}")
            for m in range(8):
                def f_mm(m=m):
                    po = ps_aux.tile([P, 256], F32, tag="aux",
                                     name=f"po{off}_{m}")
                    for kt in range(2):
                        nc.tensor.matmul(
                            po[:, 0:w], wo_sb[:, kt, 128 * m:128 * m + 128],
                            ctxT_sb[:, kt, off:off + w],
                            start=(kt == 0), stop=(kt == 1))
                    nc.vector.tensor_copy(ob[:, m, 0:w], po[:, 0:w])
                lateq.append(f_mm)

            def f_dma():
                nc.sync.dma_start(
                    outT.rearrange("(m p) t -> p m t", p=P)
                    [:, :, off:off + w], ob[:, :, 0:w])
            lateq.append(f_dma)

        # ---------- schedule ----------
        def steps_of(qb):
            nkb = qb + 1 if causal else 16
            return [(qb, kb) for kb in range(nkb)]

        items = [it for qb in range(16) for it in steps_of(qb)]
        prs = [tuple(items[i:i + 2]) for i in range(0, len(items), 2)]

        if causal:
            # group 0's projections must run up front; later groups' chains
            # are dispersed between attention steps (aux pool) so the Act
            # engine's exp stream never starves while the PE does proj work
            for ch in group_chains(0, ps_sc, "sc"):
                ch()
                drain(1)
            for nch in range(1, 4):
                for ch in group_chains(nch, ps_aux, "aux"):
                    projq.append((nch, ch))
            for pr in prs:
                force_proj(max(qb // 4 for qb, _ in pr))
                emit_step(list(pr))
        else:
            for nch in range(4):
                for ch in group_chains(nch, ps_sc, "sc"):
                    ch()
                    drain(1)
            for pr in prs:
                emit_step(list(pr))

        while pending:
            emit_ctx(*pending.popleft())
        while fillers:
            fillers.popleft()()
        while lateq:
            lateq.popleft()()

    nc.compile()
    return nc


_CACHE = {}


def _get_nc(causal: bool):
    key = bool(causal)
    if key not in _CACHE:
        _CACHE[key] = _emit(key)
    return _CACHE[key]


def _mask():
    """Multiplicative causal mask for a diagonal 128x128 block, replicated
    across the 2 head slots that share a 256-col region."""
    i = np.arange(128)[:, None]
    j = np.arange(128)[None, :]
    m = (j >= i).astype(np.float32)
    return np.tile(m, (1, 2)).astype(NPBF)


def kernel(**inputs):
    x = np.asarray(inputs["x"], dtype=np.float32)
    Wq = np.asarray(inputs["Wq"], dtype=np.float32)
    bq = np.asarray(inputs["bq"], dtype=np.float32)
    Wk = np.asarray(inputs["Wk"], dtype=np.float32)
    bk = np.asarray(inputs["bk"], dtype=np.float32)
    Wv = np.asarray(inputs["Wv"], dtype=np.float32)
    bv = np.asarray(inputs["bv"], dtype=np.float32)
    Wo = np.asarray(inputs["Wo"], dtype=np.float32)
    bo = np.asarray(inputs["bo"], dtype=np.float32)
    causal = bool(int(np.asarray(inputs["enable_causal"])))

    scale = np.float32(1.0 / np.sqrt(HD))
    wqT = Wq.T                    # [in, out]
    wkT = (Wk * scale).T
    wvT = Wv.T
    woT = Wo.T                    # [ctx-dim, out]
    bks = bk * scale

    xs = []
    for b in range(B):
        xt = x[b].T.reshape(8, 128, 4, 512).transpose(2, 1, 0, 3)
        xs.append(np.ascontiguousarray(xt.astype(NPBF)))

    ident = np.eye(P, dtype=NPBF)
    if causal:
        md = _mask()

    nc = _get_nc(causal)
    in_maps = []
    for c in range(NC):
        b, g = divmod(c, 4)
        cols = slice(256 * g, 256 * g + 256)
        wq_r = np.ascontiguousarray(
            wqT[:, cols].reshape(8, 128, 256).transpose(1, 0, 2)).astype(NPBF)
        wk_r = np.ascontiguousarray(
            wkT[:, cols].reshape(8, 128, 256).transpose(1, 0, 2)).astype(NPBF)
        wv_r = np.ascontiguousarray(
            wvT[:, cols].reshape(8, 128, 256).transpose(1, 0, 2)).astype(NPBF)
        wo_r = np.ascontiguousarray(
            woT[cols, :].reshape(2, 128, 1024).transpose(1, 0, 2)).astype(NPBF)
        m = {"xT4": xs[b],
             "wq": wq_r, "wk": wk_r, "wv": wv_r, "wo": wo_r,
             "bq": np.ascontiguousarray(bq[cols].reshape(2, 128).T),
             "bk": np.ascontiguousarray(bks[cols].reshape(2, 128).T),
             "bv": np.ascontiguousarray(bv[cols].reshape(1, 256)).astype(NPBF),
             "ident": ident}
        if causal:
            m["md"] = md
        in_maps.append(m)

    global LAST_RESULT
    res = run_bass_kernel_spmd(nc, in_maps, list(range(NC)), trace=TRACE)
    LAST_RESULT = res

    # unshard: sum the 4 head-group partials per batch (row-parallel Wo), +bo
    out = np.empty((B, S, D), dtype=np.float32)
    for b in range(B):
        acc = res.results[4 * b]["outT"].astype(np.float32)
        for g in range(1, 4):
            acc = acc + res.results[4 * b + g]["outT"]
        out[b] = acc.T + bo[None, :]
    return out


# revision 57
# speedup vs baseline: 5.9159x; 1.0045x over previous
"""Trainium2 Bass kernel for nn_MultiHeadAttention (B=2, S=2048, D=1024, H=16, causal).

Sharding across 8 NeuronCores (single SPMD program, head-parallel TP):
  - Core c owns batch b=c//4 and head group g=c%4 (4 heads = 256 of the 1024
    projection columns).  W_q/W_k/W_v are column-sharded, W_o row-sharded.
  - Each core projects Q/K/V for ALL 2048 tokens of its batch but only its 4
    heads, runs full causal attention for those heads entirely in SBUF (no
    K/V exchange => ZERO collectives), then computes its partial output
    projection out_partial = ctx_heads @ Wo_rows.  The host unshard step sums
    the 4 partial outputs per batch and adds bo (the row-parallel reduction
    of tensor-parallel attention, folded into the host-side gather that the
    full-IO contract already requires).
  - bf16 operands everywhere on the PE (1 cycle/row at any N); f32 PSUM
    accumulation; 1/sqrt(64) folded into Wk/bk on the host.
  - Attention runs as a flat software-pipelined stream of (qb, kb-pair)
    steps at 128-query granularity: transposed scores scoresT[k, q] for all
    4 heads of two key-blocks land in one [128,1024] PSUM tile (parity-
    grouped so each PSUM bank only sees one PE tile row position - HW
    constraint), a single Act exp covers the pair, causal masking is
    multiplicative on the exp'd tile (diagonal blocks only), and the
    context matmuls are q-major (stationary = exp tile, moving = V plus a
    ones-column that yields the softmax denominator as column 64).  The
    denominator is then a per-partition scalar, so normalization is plain
    tensor_scalar multiplies; a PE transpose packs the normalized context
    back to hd-major for the output projection.
  - ctx matmuls lag the score stream (software pipelining) and the
    normalize/transpose/output-projection work is spread as filler between
    later steps, so PE, Act and DVE stay concurrently busy.
"""
import numpy as np
import ml_dtypes

import concourse.bass as bass
import concourse.bacc as bacc
import concourse.mybir as mybir
import concourse.tile as tile
from concourse.bass_utils import run_bass_kernel_spmd

B, S, D, H, HD = 2, 2048, 1024, 16, 64
NC = 8
P = 128
F32 = mybir.dt.float32
BF16 = mybir.dt.bfloat16
NPBF = ml_dtypes.bfloat16

TRACE = False        # set True (e.g. from test.py) to capture an NTFF profile
LAST_RESULT = None   # BassKernelResults of the most recent kernel() call

LAG = 5              # ctx stream lags the score stream by this many pair-steps
K_FILL = 3           # filler items drained per pair-step


def _col(u, s):
    """Column of head-slot s, pair-position u in the [128,1024] score tile.

    Parity-grouped: bank 0 (cols 0:512) holds even heads (PE row base 0),
    bank 1 (cols 512:1024) odd heads (row base 64) - matmuls into one PSUM
    bank must share a single PE tile row position.
    """
    return (0 if s % 2 == 0 else 512) + 256 * u + (128 if s >= 2 else 0)


def _emit(causal: bool):
    nc = bacc.Bacc(trn_type="TRN2", num_devices=NC)
    fexp = mybir.ActivationFunctionType.Exp
    mult = mybir.AluOpType.mult

    # ---- per-core DRAM inputs (host pre-sharded / pre-transposed) ----
    xT4 = nc.dram_tensor("xT4", [4, P, 8, 512], BF16, kind="ExternalInput")
    wq_d = nc.dram_tensor("wq", [P, 8, 256], BF16, kind="ExternalInput")
    wk_d = nc.dram_tensor("wk", [P, 8, 256], BF16, kind="ExternalInput")
    wv_d = nc.dram_tensor("wv", [P, 8, 256], BF16, kind="ExternalInput")
    wo_d = nc.dram_tensor("wo", [P, 2, 1024], BF16, kind="ExternalInput")
    bq_d = nc.dram_tensor("bq", [P, 2], F32, kind="ExternalInput")
    bk_d = nc.dram_tensor("bk", [P, 2], F32, kind="ExternalInput")
    bv_d = nc.dram_tensor("bv", [1, 256], BF16, kind="ExternalInput")
    id_d = nc.dram_tensor("ident", [P, P], BF16, kind="ExternalInput")
    if causal:
        md_d = nc.dram_tensor("md", [P, 256], BF16, kind="ExternalInput")
    outT = nc.dram_tensor("outT", [D, S], F32, kind="ExternalOutput")

    with tile.TileContext(nc) as tc, \
         tc.tile_pool(name="const", bufs=1) as const, \
         tc.tile_pool(name="w", bufs=1) as wpool, \
         tc.tile_pool(name="big", bufs=1) as big, \
         tc.tile_pool(name="et", bufs=6) as etp, \
         tc.tile_pool(name="cq", bufs=2) as cqp, \
         tc.tile_pool(name="ob", bufs=2) as obp, \
         tc.tile_pool(name="ps_sc", bufs=2, space="PSUM") as ps_sc, \
         tc.tile_pool(name="ps_ctx", bufs=2, space="PSUM") as ps_ctx, \
         tc.tile_pool(name="ps_aux", bufs=2, space="PSUM") as ps_aux:

        # ---------- constants / weights (critical-path DMA order) ----------
        wk_sb = wpool.tile([P, 8, 256], BF16)
        nc.sync.dma_start(wk_sb[:], wk_d[:])
        bk_sb = const.tile([P, 2], F32)
        nc.sync.dma_start(bk_sb[:], bk_d[:])
        xt_sb = big.tile([P, 8, 2048], BF16)
        for o in range(8):  # per-o so the first proj chain starts ASAP
            nc.sync.dma_start(xt_sb[:, o, 0:512], xT4[0, :, o, :])
        wv_sb = wpool.tile([P, 8, 256], BF16)
        nc.sync.dma_start(wv_sb[:], wv_d[:])
        bv_sb = const.tile([1, 256], BF16)
        nc.sync.dma_start(bv_sb[:], bv_d[:])
        wq_sb = wpool.tile([P, 8, 256], BF16)
        nc.sync.dma_start(wq_sb[:], wq_d[:])
        bq_sb = const.tile([P, 2], F32)
        nc.sync.dma_start(bq_sb[:], bq_d[:])
        if causal:
            md_sb = const.tile([P, 256], BF16)
            nc.sync.dma_start(md_sb[:], md_d[:])
        id_sb = const.tile([P, P], BF16)
        nc.sync.dma_start(id_sb[:], id_d[:])
        wo_sb = wpool.tile([P, 2, 1024], BF16)
        nc.sync.dma_start(wo_sb[:], wo_d[:])
        for nch in range(1, 4):
            nc.sync.dma_start(xt_sb[:, :, 512 * nch:512 * nch + 512],
                              xT4[nch, :, :, :])

        ones_sb = const.tile([P, P], BF16)
        nc.gpsimd.memset(ones_sb[:], 1.0)
        # bv broadcast to all partitions once; folded into the V copy as a
        # DVE add instead of 16 per-tile bias matmuls
        bb_ps = ps_aux.tile([P, 256], F32, tag="aux", name="bb")
        nc.tensor.matmul(bb_ps[:], ones_sb[0:1, 0:P], bv_sb[:],
                         start=True, stop=True)
        bvb_sb = const.tile([P, 256], BF16)
        nc.vector.tensor_copy(bvb_sb[:], bb_ps[:])
        qt_sb = big.tile([P, 2, 2048], BF16)    # [hd-of-pair, hp, tokens]
        kt_sb = big.tile([P, 2, 2048], BF16)
        va_sb = big.tile([P, 16, 260], BF16)    # [key, kb, 4x(64 v + 1 one)]
        ctxT_sb = big.tile([P, 2, 2048], BF16)  # [hd-of-pair, kt, tokens]
        nc.gpsimd.memset(
            va_sb.rearrange("p k (s c) -> p k s c", c=65)[:, :, :, 64:65], 1.0)

        # ---------- emission machinery ----------
        import collections
        fillers = collections.deque()
        lateq = collections.deque()     # deferrable work (output projection)
        pending = collections.deque()   # (qb, pair, et, first, last)

        def drain(n):
            for _ in range(n):
                if fillers:
                    fillers.popleft()()

        def drain_late(n):
            for _ in range(n):
                if lateq:
                    lateq.popleft()()

        # ---------- phase pieces ----------
        # projection chains run in the aux PSUM pool so that, when dispersed
        # between attention steps, they never starve the score-tile slots
        def proj_qk(w_sb, b_sb, dst_sb, nch, hp, pool, tag):
            pt = pool.tile([P, 512], F32, tag=tag, name=f"pp{nch}_{hp}")
            for kt in range(8):
                nc.tensor.matmul(
                    pt[:], w_sb[:, kt, 128 * hp:128 * hp + 128],
                    xt_sb[:, kt, 512 * nch:512 * nch + 512],
                    start=(kt == 0), stop=(kt == 7))
            nc.vector.tensor_scalar_add(
                dst_sb[:, hp, 512 * nch:512 * nch + 512], pt[:],
                b_sb[:, hp:hp + 1])

        def proj_v(tt, pool, tag):
            pt = pool.tile([P, 256], F32, tag=tag, name=f"pv{tt}")
            for kt in range(8):
                nc.tensor.matmul(
                    pt[:], xt_sb[:, kt, 128 * tt:128 * tt + 128],
                    wv_sb[:, kt, :], start=(kt == 0), stop=(kt == 7))
            nc.vector.tensor_tensor(
                va_sb.rearrange("p k (s c) -> p k s c", c=65)[:, tt, :, 0:64],
                pt.rearrange("p (s c) -> p s c", c=64),
                bvb_sb.rearrange("p (s c) -> p s c", c=64),
                mybir.AluOpType.add)

        def group_chains(nch, pool, tag):
            """All projection chains needed by q-block group `nch`."""
            ch = [lambda hp=hp: proj_qk(wk_sb, bk_sb, kt_sb, nch, hp,
                                        pool, tag) for hp in range(2)]
            ch += [lambda tt=tt: proj_v(tt, pool, tag)
                   for tt in range(4 * nch, 4 * nch + 4)]
            ch += [lambda hp=hp: proj_qk(wq_sb, bq_sb, qt_sb, nch, hp,
                                         pool, tag) for hp in range(2)]
            return ch

        projq = collections.deque()   # (group, chain-closure)

        def drain_proj(n):
            for _ in range(n):
                if projq:
                    projq.popleft()[1]()

        def force_proj(g):
            while projq and projq[0][0] <= g:
                projq.popleft()[1]()

        ctx_tiles = {}

        def emit_step(items):
            """Scores + exp (+ diag masks) for 1-2 (qb, kb) items; items may
            span a q-block boundary so every exp covers a full pair-tile."""
            qb0, kb0 = items[0]
            sc = ps_sc.tile([P, 1024], F32, tag="sc", name=f"sc{qb0}_{kb0}")
            for u, (qb, kb) in enumerate(items):
                for s in range(4):
                    hb, hp = 64 * (s % 2), s // 2
                    nc.tensor.matmul(
                        sc[:, _col(u, s):_col(u, s) + 128],
                        kt_sb[hb:hb + 64, hp, 128 * kb:128 * kb + 128],
                        qt_sb[hb:hb + 64, hp, 128 * qb:128 * qb + 128],
                        start=True, stop=True)
            et = etp.tile([P, 1024], BF16, tag="et", name=f"et{qb0}_{kb0}")
            if len(items) == 2:
                nc.scalar.activation(et[:], sc[:], fexp)
            else:
                ap = sc.rearrange("p (b u c) -> p b u c", b=2, u=2)[:, :, 0, :]
                ep = et.rearrange("p (b u c) -> p b u c", b=2, u=2)[:, :, 0, :]
                nc.scalar.activation(ep, ap, fexp)
            if causal:
                for u, (qb, kb) in enumerate(items):
                    if kb == qb:   # diagonal block of this q-block
                        nc.vector.tensor_tensor(
                            et[:, 256 * u:256 * u + 256],
                            et[:, 256 * u:256 * u + 256], md_sb[:], mult)
                        nc.vector.tensor_tensor(
                            et[:, 512 + 256 * u:512 + 256 * u + 256],
                            et[:, 512 + 256 * u:512 + 256 * u + 256],
                            md_sb[:], mult)
            pending.append((items, et))
            if len(pending) > LAG:
                emit_ctx(*pending.popleft())
            drain(K_FILL)
            drain_proj(1)
            if items[-1][0] >= 10 and not projq:
                drain_late(3)

        def emit_ctx(items, et):
            for u, (qb, kb) in enumerate(items):
                nkb = qb + 1 if causal else 16
                if qb not in ctx_tiles:
                    ctx_tiles[qb] = ps_ctx.tile([P, 260], F32, tag="ctx",
                                                name=f"ctx{qb}")
                cx = ctx_tiles[qb]
                for s in range(4):
                    nc.tensor.matmul(
                        cx[:, 65 * s:65 * s + 65],
                        et[:, _col(u, s):_col(u, s) + 128],
                        va_sb[:, kb, 65 * s:65 * s + 65],
                        start=(kb == 0 and s == 0),
                        stop=(kb == nkb - 1 and s == 3))
                if kb == nkb - 1:
                    push_normalize(qb)

        def push_normalize(qb):
            cx = ctx_tiles[qb]
            cq = cqp.tile([P, 256], BF16, tag="cq", name=f"cq{qb}")
            recip = cqp.tile([P, 4], F32, tag="recip", name=f"rc{qb}")

            def f_recip():
                nc.vector.reciprocal(
                    recip[:],
                    cx.rearrange("p (s c) -> p s c", c=65)[:, :, 64])
            fillers.append(f_recip)
            for s in range(4):
                def f_mul(s=s):
                    nc.vector.tensor_scalar_mul(
                        cq[:, 64 * s:64 * s + 64],
                        cx[:, 65 * s:65 * s + 64], recip[:, s:s + 1])
                fillers.append(f_mul)
            for hp in range(2):
                def f_tr(hp=hp):
                    tr = ps_aux.tile([P, P], BF16, tag="aux",
                                     name=f"tr{qb}_{hp}")
                    nc.tensor.transpose(tr[:], cq[:, 128 * hp:128 * hp + 128],
                                        id_sb[:])
                    nc.vector.tensor_copy(
                        ctxT_sb[:, hp, 128 * qb:128 * qb + 128], tr[:])
                    if hp == 1:
                        # ctxT for this q-block (pair) is now fully emitted ->
                        # its output projection may be scheduled (lateq).
                        # The final pair is split per q-block so the first
                        # half drains during the last q-block's steps.
                        if qb % 2 == 1 and qb < 14:
                            push_oproj(256 * (qb // 2), 256)
                        elif qb >= 14:
                            push_oproj(128 * qb, 128)
                fillers.append(f_tr)

        def push_oproj(off, w):
            ob = obp.tile([P, 8, 256], F32, tag="ob", name=f"ob{off}")
            for m in range(8):
                def f_mm(m=m):
                    po = ps_aux.tile([P, 256], F32, tag="aux",
                                     name=f"po{off}_{m}")
                    for kt in range(2):
                        nc.tensor.matmul(
                            po[:, 0:w], wo_sb[:, kt, 128 * m:128 * m + 128],
                            ctxT_sb[:, kt, off:off + w],
                            start=(kt == 0), stop=(kt == 1))
                    nc.vector.tensor_copy(ob[:, m, 0:w], po[:, 0:w])
                lateq.append(f_mm)

            def f_dma():
                nc.sync.dma_start(
                    outT.rearrange("(m p) t -> p m t", p=P)
                    [:, :, off:off + w], ob[:, :, 0:w])
            lateq.append(f_dma)

        # ---------- schedule ----------
        def steps_of(qb):
            nkb = qb + 1 if causal else 16
            return [(qb, kb) for kb in range(nkb)]

        items = [it for qb in range(16) for it in steps_of(qb)]
        prs = [tuple(items[i:i + 2]) for i in range(0, len(items), 2)]

        if causal:
            # only group 0's K and Q chains must run up front: scores don't
            # read V, and the first ctx matmul trails by LAG steps, so the
            # group-0 V chains disperse into the first attention steps too
            for hp in range(2):
                proj_qk(wk_sb, bk_sb, kt_sb, 0, hp, ps_sc, "sc")
                drain(1)
            for hp in range(2):
                proj_qk(wq_sb, bq_sb, qt_sb, 0, hp, ps_sc, "sc")
                drain(1)
            for tt in range(4):
                projq.append((1, lambda tt=tt: proj_v(tt, ps_aux, "aux")))
            for nch in range(1, 4):
                for ch in group_chains(nch, ps_aux, "aux"):
                    projq.append((nch, ch))
            for pr in prs:
                force_proj(max(qb // 4 for qb, _ in pr))
                emit_step(list(pr))
        else:
            for nch in range(4):
                for ch in group_chains(nch, ps_sc, "sc"):
                    ch()
                    drain(1)
            for pr in prs:
                emit_step(list(pr))

        while pending:
            emit_ctx(*pending.popleft())
        while fillers:
            fillers.popleft()()
        while lateq:
            lateq.popleft()()

    nc.compile()
    return nc


_CACHE = {}


def _get_nc(causal: bool):
    key = bool(causal)
    if key not in _CACHE:
        _CACHE[key] = _emit(key)
    return _CACHE[key]


def _mask():
    """Multiplicative causal mask for a diagonal 128x128 block, replicated
    across the 2 head slots that share a 256-col region."""
    i = np.arange(128)[:, None]
    j = np.arange(128)[None, :]
    m = (j >= i).astype(np.float32)
    return np.tile(m, (1, 2)).astype(NPBF)


def kernel(**inputs):
    x = np.asarray(inputs["x"], dtype=np.float32)
    Wq = np.asarray(inputs["Wq"], dtype=np.float32)
    bq = np.asarray(inputs["bq"], dtype=np.float32)
    Wk = np.asarray(inputs["Wk"], dtype=np.float32)
    bk = np.asarray(inputs["bk"], dtype=np.float32)
    Wv = np.asarray(inputs["Wv"], dtype=np.float32)
    bv = np.asarray(inputs["bv"], dtype=np.float32)
    Wo = np.asarray(inputs["Wo"], dtype=np.float32)
    bo = np.asarray(inputs["bo"], dtype=np.float32)
    causal = bool(int(np.asarray(inputs["enable_causal"])))

    scale = np.float32(1.0 / np.sqrt(HD))
    wqT = Wq.T                    # [in, out]
    wkT = (Wk * scale).T
    wvT = Wv.T
    woT = Wo.T                    # [ctx-dim, out]
    bks = bk * scale

    xs = []
    for b in range(B):
        xt = x[b].T.reshape(8, 128, 4, 512).transpose(2, 1, 0, 3)
        xs.append(np.ascontiguousarray(xt.astype(NPBF)))

    ident = np.eye(P, dtype=NPBF)
    if causal:
        md = _mask()

    nc = _get_nc(causal)
    in_maps = []
    for c in range(NC):
        b, g = divmod(c, 4)
        cols = slice(256 * g, 256 * g + 256)
        wq_r = np.ascontiguousarray(
            wqT[:, cols].reshape(8, 128, 256).transpose(1, 0, 2)).astype(NPBF)
        wk_r = np.ascontiguousarray(
            wkT[:, cols].reshape(8, 128, 256).transpose(1, 0, 2)).astype(NPBF)
        wv_r = np.ascontiguousarray(
            wvT[:, cols].reshape(8, 128, 256).transpose(1, 0, 2)).astype(NPBF)
        wo_r = np.ascontiguousarray(
            woT[cols, :].reshape(2, 128, 1024).transpose(1, 0, 2)).astype(NPBF)
        m = {"xT4": xs[b],
             "wq": wq_r, "wk": wk_r, "wv": wv_r, "wo": wo_r,
             "bq": np.ascontiguousarray(bq[cols].reshape(2, 128).T),
             "bk": np.ascontiguousarray(bks[cols].reshape(2, 128).T),
             "bv": np.ascontiguousarray(bv[cols].reshape(1, 256)).astype(NPBF),
             "ident": ident}
        if causal:
            m["md"] = md
        in_maps.append(m)

    global LAST_RESULT
    res = run_bass_kernel_spmd(nc, in_maps, list(range(NC)), trace=TRACE)
    LAST_RESULT = res

    # unshard: sum the 4 head-group partials per batch (row-parallel Wo), +bo
    out = np.empty((B, S, D), dtype=np.float32)
    for b in range(B):
        acc = res.results[4 * b]["outT"].astype(np.float32)
        for g in range(1, 4):
            acc = acc + res.results[4 * b + g]["outT"]
        out[b] = acc.T + bo[None, :]
    return out
